# revision 68
# baseline (speedup 1.0000x reference)
"""DistSAGE 3-layer GraphSAGE forward on 8 TRN2 NeuronCores (Bass/Tile).

Strategy (graph/data parallel, per the DistSAGE recipe):
  - Partition the 512 seed nodes across 8 cores (64 each, LPT-balanced by
    an additive 2-hop cost estimate); build per-core dependency-driven
    blocks on the host (standard DGL block construction, pure index math).
    No inter-core communication; weights replicated.
  - Row-shard the feature table: each core receives compact per-dst-tile
    band tensors (one DRAM tensor per tile -> each load is one fully
    contiguous HBM read; a [128, W] slice of one wide tensor caps at
    ~260 GB/s).  Band tile = [dst rows TRANSPOSED, bf16] + a degree-
    sorted GRID of message rows in fp8e4: chunk k, row d = x[src]/deg of
    the k-th edge of dst d (pre-scaled on host, zero-padded).  Nodes are
    deg-sorted inside each block segment so tiles are degree-homogeneous;
    grid depth is capped at CAP0=11, excess edges spill to a few overflow
    chunks.  fp8 quantization of the (mean-aggregated) messages costs
    ~5e-3 rel err; the self path stays bf16.
  - Aggregation needs NO masks from DRAM and (for grid chunks) no mask
    generation at all: the matmul rhs is the constant identity, so
    aggT[f,d] += msgs_chunk.T @ I accumulates the pre-scaled mean in
    PSUM.  Overflow chunks use a pure one-hot rhs generated on-chip by
    one fused DVE tensor_scalar (colidx == d[p]) from 4 B/slot metadata.
    Each accumulation group owns a full PSUM bank (start=True resets the
    whole bank's has_written -- interleaved groups in one bank corrupt).
  - Y = aggT.T@W_neigh + hdT.T@W_self; dst rows arrive pre-transposed so
    no identity matmuls.  The Y matmuls of tile t-1 are emitted AFTER
    tile t's aggregation matmuls (1-deep software pipeline) so the PE
    never waits on the DVE PSUM->SBUF copy.
  - h1 is stored fp8 in DRAM.  Layer 1 gathers message rows with the
    dma_gather ucode (slot per edge, sources consumer-grouped, NI=512
    calls carrying [lo, hi) row bounds, emitted in readiness order so
    gathers stream under layer-0 compute).  Layer-1 one-hot masks are
    hoisted before the gathers (DVE does them under layer 0); h_dst
    tiles are kept in SBUF in bf16 and transposed by identity matmuls
    (transpose-DMAs on a HWDGE ring stall the whole ring FIFO on their
    h1-write dependency).
  - Layer 2 is a dense mask-matmul sweep over SBUF-resident h2 tiles.
  - DMA rings: Sync carries the band stream (+ final output), Scalar
    carries h1 writes and all consts (weights/metadata upfront; gather
    indices etc. behind the first band tile); gpsimd runs the gathers.
"""

import heapq

import numpy as np

P = 128
NCORES = 8
NUM_DST = (61952, 5632, 512)
FEAT = 256
OUTW = (256, 256, 19)
SEEDS_PER_CORE = NUM_DST[2] // NCORES  # 64
WINDOW = 32768
NI_GATHER = 512  # dma_gather indices per call (layer 1)


def _bf16():
    import ml_dtypes

    return ml_dtypes.bfloat16


def _fp8():
    import ml_dtypes

    return ml_dtypes.float8_e4m3fn


# ---------------------------------------------------------------------------
# Host-side block construction
# ---------------------------------------------------------------------------


def _balance(ids, deg, n_buckets):
    """LPT bin-packing: reorder ids so consecutive 128-groups have ~equal
    total degree (only full 128-groups are balanced)."""
    if n_buckets <= 1 or len(ids) < n_buckets * P:
        return ids
    order = np.argsort(-deg[ids], kind="stable")
    heap = [(0.0, b, 0) for b in range(n_buckets)]
    heapq.heapify(heap)
    buckets = [[] for _ in range(n_buckets)]
    for i in order:
        load, b, cnt = heapq.heappop(heap)
        buckets[b].append(ids[i])
        cnt += 1
        if cnt < P:
            heapq.heappush(heap, (load + deg[ids[i]], b, cnt))
    return np.concatenate([np.asarray(b, dtype=ids.dtype) for b in buckets])


def _seed_partition(esrc0, edst0, esrc1, edst1, esrc2, edst2, deg0, deg1):
    """LPT-balance seeds across cores by an additive 2-hop cost estimate."""
    h = np.zeros(NUM_DST[1], np.float64)
    np.add.at(h, edst1, deg0[esrc1].astype(np.float64))
    cost = np.zeros(NUM_DST[2], np.float64)
    np.add.at(cost, edst2, h[esrc2] + deg1[esrc2].astype(np.float64))
    order = np.argsort(-cost, kind="stable")
    heap = [(0.0, cc, 0) for cc in range(NCORES)]
    heapq.heapify(heap)
    groups = [[] for _ in range(NCORES)]
    for s in order:
        load, cc, cnt = heapq.heappop(heap)
        groups[cc].append(s)
        cnt += 1
        if cnt < SEEDS_PER_CORE:
            heapq.heappush(heap, (load + cost[s], cc, cnt))
    return [np.array(g, dtype=np.int64) for g in groups]


def _block_for_core(seeds, esrc0, edst0, esrc1, edst1, esrc2, edst2,
                    deg0, deg1, deg2):
    # seeds and l1_extra sorted by deg0 (ascending): layer-0 dst tiles
    # become degree-homogeneous, so the grid band layout (chunk k, row d =
    # k-th edge of dst d) pads minimally.
    seeds = seeds[np.argsort(deg0[seeds], kind="stable")]
    pos2 = np.full(NUM_DST[2], -1, np.int32)
    pos2[seeds] = np.arange(SEEDS_PER_CORE, dtype=np.int32)
    sel2 = pos2[edst2] >= 0
    es2, ed2g = esrc2[sel2], edst2[sel2]
    l1_extra = np.setdiff1d(np.unique(es2), seeds)
    l1_extra = l1_extra[np.argsort(deg0[l1_extra], kind="stable")]
    l1_out = np.concatenate([seeds, l1_extra])
    n1 = len(l1_out)

    pos1 = np.full(NUM_DST[1], -1, np.int32)
    pos1[l1_out] = np.arange(n1, dtype=np.int32)
    sel1 = pos1[edst1] >= 0
    es1, ed1g = esrc1[sel1], edst1[sel1]
    ed1 = pos1[ed1g].astype(np.int64)
    inv1 = (1.0 / np.maximum(deg1[ed1g], 1.0)).astype(np.float32)
    # Consumer-grouped l0_extra ordering: [multi-tile srcs | grp0 | grp1 ...]
    # so each layer-1 dst tile's sources sit in a contiguous band of l0 rows
    # -> its gather calls only depend on an early prefix + its own band of
    # h1, enabling overlap of the layer-1 gather under layer-0 compute.
    n1_tiles = -(-n1 // P)
    mask_x = np.ones(len(es1), bool)
    small = es1 < NUM_DST[1]
    mask_x[small] = pos1[es1[small]] < 0
    pr = np.unique(
        np.stack([es1[mask_x], ed1[mask_x] // P], axis=1), axis=0
    )
    srcs_u, first_idx, cnt = np.unique(
        pr[:, 0], return_index=True, return_counts=True
    )
    multi = srcs_u[cnt > 1]
    multi = multi[np.argsort(deg0[multi], kind="stable")]
    segs = [multi]
    single_mask = cnt == 1
    s_srcs = srcs_u[single_mask]
    s_tile = pr[first_idx[single_mask], 1]
    for tt in range(n1_tiles):
        seg = s_srcs[s_tile == tt]
        seg = seg[np.argsort(deg0[seg], kind="stable")]
        segs.append(seg)

    ed2 = pos2[ed2g].astype(np.int64)
    inv2 = (1.0 / np.maximum(deg2[ed2g], 1.0)).astype(np.float32)
    es2l = pos1[es2].astype(np.int64)

    return dict(
        seeds=seeds, l1_out=l1_out, segs=segs, n1=n1,
        e1g=(es1, ed1, inv1),
        e2=(es2l, ed2, inv2),
    )


def _assemble_l0(blocks, esrc0, edst0, deg0):
    """Build per-core l0_out = [l1_out | segs...] (tight packing), padded
    to a common n0_pad with duplicates of row 0 (they carry no edges).
    Fills blocks[c]['l0_out'], 'n0', 'e0' (local), 'e1' (local srcs)."""
    n1_pad = max(-(-b["n1"] // P) for b in blocks) * P
    n0_pad = -(-max(
        b["n1"] + sum(len(s) for s in b["segs"]) for b in blocks
    ) // P) * P
    for b in blocks:
        cat = np.concatenate([b["l1_out"]] + b["segs"])
        l0 = np.zeros(n0_pad, np.int64)
        l0[: len(cat)] = cat
        l0[len(cat) :] = cat[0]
        pos0 = np.full(NUM_DST[0], -1, np.int32)
        pos0[cat] = np.arange(len(cat), dtype=np.int32)
        b["l0_out"] = l0
        b["n0"] = n0_pad

        sel0 = pos0[edst0] >= 0
        es0, ed0g = esrc0[sel0], edst0[sel0]
        ed0 = pos0[ed0g].astype(np.int64)
        inv0 = (1.0 / np.maximum(deg0[ed0g], 1.0)).astype(np.float32)
        b["e0"] = (es0.astype(np.int64), ed0, inv0)
        es1, ed1, inv1 = b["e1g"]
        b["e1"] = (pos0[es1].astype(np.int64), ed1, inv1)
    return n0_pad, n1_pad


def _slots_by_tile(es, ed, inv, n_tiles):
    """Slot-per-edge: per dst tile, edge slots sorted by src row.
    Returns per-tile (srcs, dloc, inv) arrays (dloc = dst index in tile)."""
    tile = ed // P
    order = np.lexsort((es, tile))
    es, ed, inv, tile = es[order], ed[order], inv[order], tile[order]
    starts = np.searchsorted(tile, np.arange(n_tiles))
    ends = np.searchsorted(tile, np.arange(n_tiles) + 1)
    return [
        (es[s:e], (ed[s:e] - t * P).astype(np.int64), inv[s:e])
        for t, (s, e) in enumerate(zip(starts, ends))
    ]


CAP0 = 11  # grid chunks per tile cap; excess edges go to overflow chunks


def _grid_by_tile(es, ed, inv, n_tiles, cap):
    """Grid layout: per dst tile, edge slot = rank*128 + dloc (rank = edge
    index within its dst), capped at `cap` ranks; excess edges spill to an
    overflow list (slot-per-edge with one-hot masks)."""
    tile = ed // P
    order = np.lexsort((es, ed))
    es, ed, inv = es[order], ed[order], inv[order]
    tile = tile[order]
    starts = np.searchsorted(tile, np.arange(n_tiles))
    ends = np.searchsorted(tile, np.arange(n_tiles) + 1)
    out = []
    for t, (s, e) in enumerate(zip(starts, ends)):
        dloc = (ed[s:e] - t * P).astype(np.int64)
        cnt = np.bincount(dloc, minlength=P)
        first = np.concatenate([[0], np.cumsum(cnt)[:-1]])
        rank = np.arange(e - s) - first[dloc]  # edges sorted by dloc
        ing = rank < cap
        out.append(dict(
            gsrc=es[s:e][ing], gslot=rank[ing] * P + dloc[ing],
            ginv=inv[s:e][ing],
            K=int(min(cnt.max(), cap)) if e > s else 0,
            osrc=es[s:e][~ing], od=dloc[~ing], oinv=inv[s:e][~ing],
        ))
    return out


class GatherPlan:
    """Layer 1: continuous slot stream gathered via dma_gather (one slot
    per edge).  Tile t owns stream slots [slot_off[t], slot_off[t]+m[t]);
    chunks are 128-slot groups; a chunk overlapping two tiles gets one
    metadata column per tile.  Calls are tile-aligned (big calls + a small
    tail call per tile) and carry [lo, hi) row bounds so each call only
    depends on the h-table rows it reads."""

    def __init__(self, n_tiles, slot_counts, ni):
        self.ni = ni
        self.cpc = ni // P  # max chunks per big call
        self.n_tiles = n_tiles
        self.m = slot_counts
        self.slot_off = np.concatenate([[0], np.cumsum(slot_counts)]).astype(np.int64)
        total = int(self.slot_off[-1])
        self.n_chunks = -(-total // P)
        self.n_chunks_pad = self.n_chunks
        # tile-aligned call partition: big calls + small tail call per tile
        TAILC = 2
        self.call_sizes = []
        for t in range(n_tiles):
            s = -(-int(self.slot_off[t]) // P)
            e = -(-int(self.slot_off[t + 1]) // P) if t + 1 < n_tiles else self.n_chunks
            if t + 1 == n_tiles:
                e = self.n_chunks
            nch = e - s
            if nch <= 0:
                continue
            if nch > TAILC + 1:
                head = nch - TAILC
                nbig = -(-head // self.cpc)
                base = head // nbig
                rem = head - base * nbig
                self.call_sizes += [base + (1 if i < rem else 0)
                                    for i in range(nbig)]
                self.call_sizes.append(TAILC)
            else:
                self.call_sizes.append(nch)
        assert sum(self.call_sizes) == self.n_chunks
        self.n_calls = len(self.call_sizes)
        self.call_chunk_off = np.concatenate(
            [[0], np.cumsum(self.call_sizes)]
        ).astype(np.int64)
        self.pairs = []
        self.tile_pairs = []  # per tile: list of (sp_col, chunk)
        for t in range(n_tiles):
            lo, hi = int(self.slot_off[t]), int(self.slot_off[t + 1])
            ch1 = (hi - 1) // P if hi > lo else lo // P
            tp = []
            for ch in range(lo // P, ch1 + 1):
                tp.append((len(self.pairs), ch))
                self.pairs.append((t, ch))
            self.tile_pairs.append(tp)
        self.n_sp_cols = len(self.pairs)
        self.gidx = []  # [NCORES][128, n_chunks_pad] int64 table rows
        self.md = []  # [NCORES][128, n_sp_cols*2] f32 (d, inv) per slot
        self.call_base = None  # [n_calls] row base (lo) per call
        self.call_hi = None  # [n_calls] exclusive row bound per call

    def compute_call_bounds(self, nrows):
        """Per-call [lo, hi) over all cores, 128-aligned."""
        lo = np.zeros(self.n_calls, np.int64)
        hi = np.zeros(self.n_calls, np.int64)
        for k in range(self.n_calls):
            a, b = int(self.call_chunk_off[k]), int(self.call_chunk_off[k + 1])
            mn, mx = nrows, 0
            for g in self.gidx:
                sl = g[:, a:b]
                mn = min(mn, int(sl.min()))
                mx = max(mx, int(sl.max()))
            lo[k] = (mn // P) * P
            hi[k] = min(nrows, ((mx // P) + 1) * P)
        self.call_base = lo
        self.call_hi = hi


def _fill_gather(plan, per_core_tiles, pad_row):
    total_pad = plan.n_chunks_pad * P
    for c in range(NCORES):
        stream = np.zeros(total_pad, np.int64)
        dstream = np.full(total_pad, -1.0, np.float32)
        istream = np.zeros(total_pad, np.float32)
        for t in range(plan.n_tiles):
            lo, hi = int(plan.slot_off[t]), int(plan.slot_off[t + 1])
            srcs, dloc, inv = per_core_tiles[c][t]
            stream[lo : lo + len(srcs)] = srcs
            stream[lo + len(srcs) : hi] = pad_row[c][t]
            dstream[lo : lo + len(srcs)] = dloc
            istream[lo : lo + len(srcs)] = inv
        tail = int(plan.slot_off[-1])
        stream[tail:] = pad_row[c][plan.n_tiles - 1]
        plan.gidx.append(stream.reshape(plan.n_chunks_pad, P).T.copy())

        # metadata: per (tile, chunk) pair one (d, inv) column; slots of
        # the chunk outside the tile's range get (-1, 0).
        md = np.zeros((P, plan.n_sp_cols, 2), np.float32)
        md[:, :, 0] = -1.0
        for t in range(plan.n_tiles):
            lo, hi = int(plan.slot_off[t]), int(plan.slot_off[t + 1])
            for sp_col, ch in plan.tile_pairs[t]:
                s0 = ch * P
                a = max(s0, lo)
                b = min(s0 + P, hi)
                if a < b:
                    md[a - s0 : b - s0, sp_col, 0] = dstream[a:b]
                    md[a - s0 : b - s0, sp_col, 1] = istream[a:b]
        plan.md.append(np.ascontiguousarray(md.reshape(P, plan.n_sp_cols * 2)))


class BandPlan:
    """Layer 0: per-tile dense bands, pre-interleaved.  Tile t's band =
    group 0 (dst rows TRANSPOSED: [f0..127 x d | f128..255 x d]) +
    K[t] grid groups (chunk k row d = x[src]/deg of the k-th edge of dst
    d, pre-scaled on host, zero-padded -> aggregation rhs is the constant
    identity) + O[t] overflow groups (slot-per-edge, pre-scaled, one-hot
    is_equal masks from per-slot dst metadata)."""

    def __init__(self, n_tiles, K, O):
        self.n_tiles = n_tiles
        self.K = [max(1, k) for k in K]  # capped max-over-core degree
        self.O = O  # overflow chunks per tile (max over cores)
        self.goff = np.concatenate(
            [[0], np.cumsum([1 + k + o for k, o in zip(self.K, O)])]
        ).astype(np.int64)
        self.n_groups = int(self.goff[-1])
        self.ov_off = np.concatenate([[0], np.cumsum(O)]).astype(np.int64)
        self.n_ov_cols = int(self.ov_off[-1])
        self.mdov = []  # [NCORES][128, n_ov_cols] f32 dst-id per slot


def build_host(inputs):
    esrc0 = np.asarray(inputs["esrc0"]).astype(np.int64)
    edst0 = np.asarray(inputs["edst0"]).astype(np.int64)
    esrc1 = np.asarray(inputs["esrc1"]).astype(np.int64)
    edst1 = np.asarray(inputs["edst1"]).astype(np.int64)
    esrc2 = np.asarray(inputs["esrc2"]).astype(np.int64)
    edst2 = np.asarray(inputs["edst2"]).astype(np.int64)
    x = np.asarray(inputs["x"], dtype=np.float32)

    deg0 = np.bincount(edst0, minlength=NUM_DST[0]).astype(np.float32)
    deg1 = np.bincount(edst1, minlength=NUM_DST[1]).astype(np.float32)
    deg2 = np.bincount(edst2, minlength=NUM_DST[2]).astype(np.float32)

    seed_groups = _seed_partition(esrc0, edst0, esrc1, edst1, esrc2, edst2,
                                  deg0, deg1)
    blocks = [
        _block_for_core(seed_groups[c], esrc0, edst0, esrc1, edst1, esrc2,
                        edst2, deg0, deg1, deg2)
        for c in range(NCORES)
    ]
    n0_pad, n1_pad = _assemble_l0(blocks, esrc0, edst0, deg0)
    T0, T1, T2 = n0_pad // P, n1_pad // P, 1

    tiles0 = [_grid_by_tile(*b["e0"], T0, CAP0) for b in blocks]
    tiles1 = [_slots_by_tile(*b["e1"], T1) for b in blocks]

    # ---- layer 0: band plan + pre-interleaved compact tables ----
    plan0 = BandPlan(
        T0,
        [max(tiles0[c][t]["K"] for c in range(NCORES)) for t in range(T0)],
        [max(-(-len(tiles0[c][t]["osrc"]) // P) for c in range(NCORES))
         for t in range(T0)],
    )
    l0_padded = [b["l0_out"] for b in blocks]

    bf16 = _bf16()
    fp8 = _fp8()
    x16 = x.astype(bf16)
    # per-core, per-tile: dst groups (bf16, transposed) + msg grids (fp8)
    bandd, bandm = [], []
    for c in range(NCORES):
        bd, bm = [], []
        mdov = np.full((P, max(plan0.n_ov_cols, 1)), -1.0, np.float32)
        for t in range(T0):
            hd = x16[l0_padded[c][t * P : (t + 1) * P]]  # [128 dst, 256 f]
            bd.append(np.ascontiguousarray(
                hd.T.reshape(2, P, P).transpose(1, 0, 2).reshape(P, FEAT)
            ))
            # bd[t][p, h*128+j] = hd[j, h*128+p]
            ti = tiles0[c][t]
            K, O = plan0.K[t], plan0.O[t]
            grid = np.zeros(((K + O) * P, FEAT), np.float32)
            grid[ti["gslot"]] = x[ti["gsrc"]] * ti["ginv"][:, None]
            if O:
                no = len(ti["osrc"])
                grid[K * P : K * P + no] = x[ti["osrc"]] * ti["oinv"][:, None]
                oo = int(plan0.ov_off[t])
                dcol = np.full(O * P, -1.0, np.float32)
                dcol[:no] = ti["od"]
                mdov[:, oo : oo + O] = dcol.reshape(O, P).T
            g8 = grid.astype(fp8).reshape(K + O, P, FEAT)
            bm.append(np.ascontiguousarray(
                g8.transpose(1, 0, 2).reshape(P, (K + O) * FEAT)
            ))
        bandd.append(bd)
        bandm.append(bm)
        plan0.mdov.append(np.ascontiguousarray(mdov))

    # ---- layer 1: gather plan (slot per edge) ----
    m1 = [
        max(1, max(len(tiles1[c][t][0]) for c in range(NCORES)))
        for t in range(T1)
    ]
    plan1 = GatherPlan(T1, m1, NI_GATHER)
    padL = [[t * P for t in range(T1)] for _ in range(NCORES)]
    _fill_gather(plan1, tiles1, padL)
    plan1.compute_call_bounds(n0_pad)
    assert n0_pad <= WINDOW and n1_pad <= WINDOW

    # ---- layer 2: dense sweep over SBUF-resident h2 (no gather) ----
    # sp2dense[c][j] = [128, 128] mask: W[row, seed] = sum inv2 over edges
    # (src local j*128+row -> seed).
    sp2d = []
    for c in range(NCORES):
        es, ed, inv = blocks[c]["e2"]
        W = np.zeros((T1, P, P), np.float32)
        np.add.at(W, (es // P, es % P, ed), inv)
        sp2d.append(
            np.ascontiguousarray(
                W.transpose(1, 0, 2).reshape(P, T1 * P).astype(bf16)
            )
        )

    return dict(
        plan0=plan0,
        plans=(plan1,),
        sp2d=sp2d,
        T=(T0, T1, T2),
        n0_pad=n0_pad,
        n1_pad=n1_pad,
        bandd=bandd,
        bandm=bandm,
        blocks=blocks,
        weights=tuple(
            (
                np.asarray(inputs[f"W_self{l}"], np.float32),
                np.asarray(inputs[f"W_neigh{l}"], np.float32),
                np.asarray(inputs[f"b{l}"], np.float32),
            )
            for l in range(3)
        ),
    )


# ---------------------------------------------------------------------------
# Numpy simulation of the device kernel (validation aid; fp32 stand-in)
# ---------------------------------------------------------------------------


def simulate_core(meta, c):
    plan0 = meta["plan0"]
    colidx = np.arange(P, dtype=np.float32)

    mdov = plan0.mdov[c]
    ws, wn, b = meta["weights"][0]
    table = np.zeros((plan0.n_tiles * P, OUTW[0]), np.float32)
    for t in range(plan0.n_tiles):
        K, O = plan0.K[t], plan0.O[t]
        hdT = meta["bandd"][c][t].astype(np.float32)
        hd = np.concatenate([hdT[:, 0:P].T, hdT[:, P : 2 * P].T], axis=1)
        xm = meta["bandm"][c][t].astype(np.float32).reshape(P, K + O, FEAT)
        mean = xm[:, 0:K, :].sum(axis=1)  # [d, f]
        oo = int(plan0.ov_off[t])
        for o in range(O):
            msgs = xm[:, K + o, :]
            onehot = colidx[None, :] == mdov[:, oo + o : oo + o + 1]
            mean += (msgs.T @ onehot).T
        table[t * P : (t + 1) * P] = np.maximum(hd @ ws + mean @ wn + b, 0.0)

    plan = meta["plans"][0]
    md1 = plan.md[c].reshape(P, plan.n_sp_cols, 2)
    table8 = table.astype(_fp8()).astype(np.float32)  # h1buf is fp8
    ws, wn, b = meta["weights"][1]
    out = np.zeros((plan.n_tiles * P, OUTW[1]), np.float32)
    for t in range(plan.n_tiles):
        hd = table[t * P : (t + 1) * P]
        aggT = np.zeros((FEAT, P), np.float32)
        for sp_col, ch in plan.tile_pairs[t]:
            msgs = table8[plan.gidx[c][:, ch]]
            mask = (colidx[None, :] == md1[:, sp_col, 0:1]) * md1[:, sp_col, 1:2]
            aggT += msgs.T @ mask
        out[t * P : (t + 1) * P] = np.maximum(hd @ ws + aggT.T @ wn + b, 0.0)
    table = out

    # layer 2: dense sweep
    ws, wn, b = meta["weights"][2]
    sp2 = meta["sp2d"][c].astype(np.float32).reshape(P, -1, P)
    hd = table[0:P]
    aggT = np.zeros((FEAT, P), np.float32)
    for j in range(sp2.shape[1]):
        aggT += table[j * P : (j + 1) * P].T @ sp2[:, j, :]
    y = hd @ ws + aggT.T @ wn + b
    return y[:SEEDS_PER_CORE]


# ---------------------------------------------------------------------------
# Device kernel
# ---------------------------------------------------------------------------


def _wrap_idx16(plan, c):
    bases = np.zeros(plan.n_chunks_pad, np.int64)
    for k in range(plan.n_calls):
        bases[plan.call_chunk_off[k] : plan.call_chunk_off[k + 1]] = plan.call_base[k]
    rel = plan.gidx[c] - bases[None, :]
    total16 = plan.n_chunks_pad * P // 16
    out = np.zeros((P, total16), np.int16)
    off16 = 0
    for k in range(plan.n_calls):
        a, b = int(plan.call_chunk_off[k]), int(plan.call_chunk_off[k + 1])
        flat = rel[:, a:b].T.reshape(-1)
        w = flat.reshape(len(flat) // 16, 16).T.astype(np.int16)
        out[:16, off16 : off16 + w.shape[1]] = w
        off16 += w.shape[1]
    for rep in range(1, 8):
        out[rep * 16 : (rep + 1) * 16] = out[:16]
    return out


def run_device(meta, trace=False, debug_h1=False):
    import concourse.bacc as bacc
    import concourse.tile as tile
    import concourse.mybir as mybir
    from concourse.bass_utils import run_bass_kernel_spmd

    plan0 = meta["plan0"]
    plan1 = meta["plans"][0]
    T1 = meta["T"][1]
    f32 = mybir.dt.float32
    i32 = mybir.dt.int32
    b16 = mybir.dt.bfloat16
    f8 = mybir.dt.float8e4
    alu = mybir.AluOpType

    nc = bacc.Bacc("TRN2", target_bir_lowering=False, debug=False, num_devices=NCORES)

    # one DRAM tensor per band tile: the transfer is then one fully
    # contiguous HBM region (a [P, W] slice of a wide tensor reads 128
    # scattered ~7KB segments and caps at ~260 GB/s).  dst groups bf16,
    # message grids fp8 (mean-aggregated -> quantization error washes out)
    bandd_d = [
        nc.dram_tensor(f"bandd{t}", [P, FEAT], b16, kind="ExternalInput")
        for t in range(plan0.n_tiles)
    ]
    bandm_d = [
        nc.dram_tensor(
            f"bandm{t}",
            [P, (plan0.K[t] + plan0.O[t]) * FEAT],
            f8,
            kind="ExternalInput",
        )
        for t in range(plan0.n_tiles)
    ]
    mdov_d = nc.dram_tensor("mdov", [P, max(plan0.n_ov_cols, 1)], f32,
                            kind="ExternalInput")
    ident_d = nc.dram_tensor("ident", [P, P], b16, kind="ExternalInput")
    ones_d = nc.dram_tensor("ones", [1, P], b16, kind="ExternalInput")
    h1buf = nc.dram_tensor("h1buf", [meta["n0_pad"], FEAT], f8)
    out_d = nc.dram_tensor("out", [SEEDS_PER_CORE, OUTW[2]], f32, kind="ExternalOutput")

    h1o_d = None
    if debug_h1:
        h1o_d = nc.dram_tensor("h1o", [meta["n0_pad"], FEAT], f32,
                               kind="ExternalOutput")
    idx1_d = nc.dram_tensor("gidx1", [P, plan1.n_chunks_pad * P // 16],
                            mybir.dt.int16, kind="ExternalInput")
    md1_d = nc.dram_tensor("md1", [P, plan1.n_sp_cols * 2], f32,
                           kind="ExternalInput")
    sp2_d = nc.dram_tensor("sp2d", [P, T1 * P], b16, kind="ExternalInput")
    w_d = []
    for l in range(3):
        w_d.append(
            (
                nc.dram_tensor(f"ws{l}", [FEAT, OUTW[l]], b16, kind="ExternalInput"),
                nc.dram_tensor(f"wn{l}", [FEAT, OUTW[l]], b16, kind="ExternalInput"),
                nc.dram_tensor(f"bias{l}", [1, OUTW[l]], b16, kind="ExternalInput"),
            )
        )

    use_bias = [bool(np.any(meta["weights"][l][2] != 0)) for l in range(3)]

    with tile.TileContext(nc) as tc:
        with (
            tc.tile_pool(name="const", bufs=1) as cpool,
            tc.tile_pool(name="msgs", bufs=12) as mpool,
            tc.tile_pool(name="dsts", bufs=12) as dpool,
            tc.tile_pool(name="mask", bufs=28) as kpool,
            tc.tile_pool(name="acc", bufs=3) as apool,
            tc.tile_pool(name="outp", bufs=3) as opool,
            tc.tile_pool(name="hdt", bufs=3) as hpool,
            tc.tile_pool(name="gmsg", bufs=1) as gpool,
            tc.tile_pool(name="pagg", bufs=2, space="PSUM") as pa,
            tc.tile_pool(name="py", bufs=2, space="PSUM") as pypool,
        ):
            # ---- upfront consts ----
            ident_t = cpool.tile([P, P], b16, tag="ident")
            nc.scalar.dma_start(out=ident_t[:], in_=ident_d[:])
            mdov_t = cpool.tile([P, max(plan0.n_ov_cols, 1)], f32, tag="mdov")
            nc.scalar.dma_start(out=mdov_t[:], in_=mdov_d[:])
            md1_t = cpool.tile([P, plan1.n_sp_cols * 2], f32, tag="md1")
            nc.scalar.dma_start(out=md1_t[:], in_=md1_d[:])
            colidx_i = cpool.tile([P, P], i32, tag="colidx_i")
            nc.gpsimd.iota(colidx_i[:], [[1, P]], channel_multiplier=0)
            colidx = cpool.tile([P, P], f32, tag="colidx")
            nc.vector.tensor_copy(out=colidx[:], in_=colidx_i[:])

            ws_ts, wn_ts, bias_ts = [[None, None] for _ in range(3)], \
                [[None, None] for _ in range(3)], [None] * 3
            ones_t = cpool.tile([1, P], b16, tag="ones")

            def load_weights(l, eng):
                outw = OUTW[l]
                for k in range(2):
                    w = cpool.tile([P, outw], b16, tag=f"ws{l}_{k}")
                    eng.dma_start(out=w[:], in_=w_d[l][0][k * P : (k + 1) * P, :])
                    ws_ts[l][k] = w
                    w = cpool.tile([P, outw], b16, tag=f"wn{l}_{k}")
                    eng.dma_start(out=w[:], in_=w_d[l][1][k * P : (k + 1) * P, :])
                    wn_ts[l][k] = w
                if use_bias[l]:
                    bias_t = cpool.tile([1, outw], b16, tag=f"bias{l}")
                    eng.dma_start(out=bias_t[:], in_=w_d[l][2][:])
                    bias_ts[l] = bias_t

            load_weights(0, nc.scalar)
            if any(use_bias):
                nc.scalar.dma_start(out=ones_t[:], in_=ones_d[:])

            h2res = [
                cpool.tile([P, FEAT], b16, tag=f"h2res_{t}", name=f"h2res_{t}")
                for t in range(T1)
            ]
            h1res = [
                cpool.tile([P, FEAT], b16, tag=f"h1res_{t}", name=f"h1res_{t}")
                for t in range(T1)
            ]

            def gen_mask(md_t, col):
                """One-hot mask [128 slots, 128 dst] = (colidx==d[p]) * inv[p]."""
                mk = kpool.tile([P, P], b16, tag="mk")
                nc.vector.tensor_scalar(
                    out=mk[:],
                    in0=colidx[:],
                    scalar1=md_t[:, 2 * col : 2 * col + 1],
                    scalar2=md_t[:, 2 * col + 1 : 2 * col + 2],
                    op0=alu.is_equal,
                    op1=alu.mult,
                )
                return mk

            def gen_mask_ov(col):
                """Pure one-hot [128 slots, 128 dst] = (colidx==d[p])."""
                mk = kpool.tile([P, P], b16, tag="mk")
                nc.vector.tensor_scalar(
                    out=mk[:],
                    in0=colidx[:],
                    scalar1=mdov_t[:, col : col + 1],
                    scalar2=None,
                    op0=alu.is_equal,
                )
                return mk

            def tile_tail(l, t, ac, hdT, dest):
                """Y matmuls + bias + activation + store for one dst tile.
                ac = aggT halves [f-half, d] bf16; hdT = dst rows transposed."""
                outw = OUTW[l]
                y = pypool.tile([P, outw], f32, tag="y")
                nc.tensor.matmul(y[:], lhsT=ac[:, 0:P], rhs=wn_ts[l][0][:],
                                 start=True, stop=False)
                nc.tensor.matmul(y[:], lhsT=ac[:, P : 2 * P], rhs=wn_ts[l][1][:],
                                 start=False, stop=False)
                nc.tensor.matmul(y[:], lhsT=hdT[:, 0:P], rhs=ws_ts[l][0][:],
                                 start=False, stop=False)
                nc.tensor.matmul(y[:], lhsT=hdT[:, P : 2 * P], rhs=ws_ts[l][1][:],
                                 start=False, stop=not use_bias[l])
                if use_bias[l]:
                    nc.tensor.matmul(y[:], lhsT=ones_t[0:1, :],
                                     rhs=bias_ts[l][0:1, :],
                                     start=False, stop=True)
                if l == 0:
                    o2 = opool.tile([P, outw], f8, tag="o2")
                    nc.scalar.activation(
                        out=o2[:], in_=y[:],
                        func=mybir.ActivationFunctionType.Relu,
                    )
                    nc.scalar.dma_start(out=dest[t * P : (t + 1) * P, :], in_=o2[:])
                    if t < T1:
                        # bf16 copy kept on-chip for the layer-1 self path
                        nc.scalar.activation(
                            out=h1res[t][:], in_=y[:],
                            func=mybir.ActivationFunctionType.Relu,
                        )
                    if debug_h1:
                        od = opool.tile([P, outw], f32, tag="od")
                        nc.vector.tensor_copy(out=od[:], in_=o2[:])
                        nc.sync.dma_start(
                            out=h1o_d[t * P : (t + 1) * P, :], in_=od[:]
                        )
                elif l == 1:
                    nc.scalar.activation(
                        out=h2res[t][:], in_=y[:],
                        func=mybir.ActivationFunctionType.Relu,
                    )
                else:
                    o = opool.tile([P, outw], f32, tag="o")
                    nc.vector.tensor_copy(out=o[:], in_=y[:])
                    nc.sync.dma_start(out=dest[:], in_=o[0:SEEDS_PER_CORE, :])

            # ================= layer 0: dense bands =================
            # 1-deep software pipeline: tile t's mask matmuls are emitted
            # before tile t-1's PSUM copy + Y matmuls, so the PE never
            # waits on the DVE copy.
            Kmax = max(k + o for k, o in zip(plan0.K, plan0.O))
            pending = None  # (t, pc0, pc1, hdT_view)
            for t in range(plan0.n_tiles):
                K, O = plan0.K[t], plan0.O[t]
                oo = int(plan0.ov_off[t])
                btd = dpool.tile([P, FEAT], b16, tag="bandd")
                nc.sync.dma_start(out=btd[:], in_=bandd_d[t][:])
                btm = mpool.tile([P, Kmax * FEAT], f8, tag="bandm")
                nc.sync.dma_start(
                    out=btm[:, : (K + O) * FEAT], in_=bandm_d[t][:]
                )
                omasks = [gen_mask_ov(oo + o) for o in range(O)]
                # two PSUM tiles: each accumulation group must own its bank
                # (start=True resets the whole bank's has_written); rhs is
                # the constant identity for grid chunks (rows pre-scaled,
                # dst-aligned), a one-hot mask for overflow chunks
                pc0 = pa.tile([P, P], f32, tag="pc0")
                pc1 = pa.tile([P, P], f32, tag="pc1")
                for k in range(K + O):
                    st, sp = (k == 0), (k == K + O - 1)
                    base = k * FEAT
                    rhs = ident_t[:] if k < K else omasks[k - K][:]
                    nc.tensor.matmul(pc0[:], lhsT=btm[:, base : base + P],
                                     rhs=rhs, start=st, stop=sp)
                    nc.tensor.matmul(pc1[:],
                                     lhsT=btm[:, base + P : base + 2 * P],
                                     rhs=rhs, start=st, stop=sp)
                if pending is not None:
                    tp, pc0p, pc1p, hdTp = pending
                    ac = apool.tile([P, FEAT], b16, tag="ac")
                    nc.vector.tensor_copy(out=ac[:, 0:P], in_=pc0p[:])
                    nc.vector.tensor_copy(out=ac[:, P : 2 * P], in_=pc1p[:])
                    tile_tail(0, tp, ac, hdTp, h1buf)
                pending = (t, pc0, pc1, btd[:])

                if t == 0:
                    # late consts: emitted behind the first band loads so the
                    # main stream starts immediately; all are ready long
                    # before their consumers run.
                    idx1_t = cpool.tile(list(idx1_d.shape), mybir.dt.int16,
                                        tag="idx1")
                    nc.scalar.dma_start(out=idx1_t[:], in_=idx1_d[:])
                    load_weights(1, nc.scalar)
                    load_weights(2, nc.scalar)
                    sp2_t = cpool.tile([P, T1 * P], b16, tag="sp2d")
                    nc.scalar.dma_start(out=sp2_t[:], in_=sp2_d[:])
            # flush the pipeline
            tp, pc0p, pc1p, hdTp = pending
            ac = apool.tile([P, FEAT], b16, tag="ac")
            nc.vector.tensor_copy(out=ac[:, 0:P], in_=pc0p[:])
            nc.vector.tensor_copy(out=ac[:, P : 2 * P], in_=pc1p[:])
            tile_tail(0, tp, ac, hdTp, h1buf)

            # ================= layer 1: overlapped gather =================
            # hoist all layer-1 mask generation (depends only on md1/colidx)
            # onto GPSIMD, which idles during the first half of layer 0 --
            # keeps the DVE free for the layer-0 PSUM copies and the masks
            # out of the gather-bound tail
            l1_masks = []
            for t in range(plan1.n_tiles):
                tm = []
                for i, (sp_col, _) in enumerate(plan1.tile_pairs[t]):
                    mk = cpool.tile([P, P], b16, tag=f"mk1_{t}_{i}",
                                    name=f"mk1_{t}_{i}")
                    nc.gpsimd.tensor_scalar(
                        out=mk[:],
                        in0=colidx[:],
                        scalar1=md1_t[:, 2 * sp_col : 2 * sp_col + 1],
                        scalar2=md1_t[:, 2 * sp_col + 1 : 2 * sp_col + 2],
                        op0=alu.is_equal,
                        op1=alu.mult,
                    )
                    tm.append(mk)
                l1_masks.append(tm)

            # transposed h_dst tiles via identity matmuls from the
            # SBUF-resident h1res copies (no DMA-ring blocking)
            hdts = []
            for t in range(T1):
                ph = pa.tile([P, FEAT], f32, tag="pht", name=f"pht_{t}")
                nc.tensor.matmul(ph[:, 0:P], lhsT=h1res[t][:, 0:P],
                                 rhs=ident_t[:], start=True, stop=True)
                nc.tensor.matmul(ph[:, P : 2 * P], lhsT=h1res[t][:, P : 2 * P],
                                 rhs=ident_t[:], start=True, stop=True)
                ht = hpool.tile([P, FEAT], b16, tag=f"hdt_{t}", name=f"hdt_{t}")
                nc.vector.tensor_copy(out=ht[:], in_=ph[:])
                hdts.append(ht)

            call_tiles = [None] * plan1.n_calls
            order = sorted(
                range(plan1.n_calls),
                key=lambda k: (int(plan1.call_hi[k]), int(plan1.call_base[k])),
            )
            for k in order:
                a = int(plan1.call_chunk_off[k])
                b2 = int(plan1.call_chunk_off[k + 1])
                sz = b2 - a
                lo = int(plan1.call_base[k])
                hi = int(plan1.call_hi[k])
                mt = gpool.tile([P, sz * FEAT], f8, tag=f"msgs1_{k}")
                nc.gpsimd.dma_gather(
                    out_ap=mt[:, : sz * FEAT].rearrange("p (g d) -> p g d", g=sz),
                    in_ap=h1buf[lo:hi, :],
                    idxs_ap=idx1_t[:, a * P // 16 : b2 * P // 16],
                    num_idxs=sz * P,
                    num_idxs_reg=sz * P,
                    elem_size=FEAT,
                    single_packet=False,
                )
                call_tiles[k] = (mt, a)

            call_of_chunk = np.searchsorted(
                plan1.call_chunk_off, np.arange(plan1.n_chunks_pad), side="right"
            ) - 1

            def msg_slice(ch, f0, f1):
                k = int(call_of_chunk[ch])
                mt, a = call_tiles[k]
                j = ch - a
                return mt[:, j * FEAT + f0 : j * FEAT + f1]

            pending = None
            for t in range(plan1.n_tiles):
                pairs = plan1.tile_pairs[t]
                masks = l1_masks[t]
                pc0 = pa.tile([P, P], f32, tag="pc0")
                pc1 = pa.tile([P, P], f32, tag="pc1")
                for i, (sp_col, ch) in enumerate(pairs):
                    st, sp = (i == 0), (i == len(pairs) - 1)
                    nc.tensor.matmul(pc0[:], lhsT=msg_slice(ch, 0, P),
                                     rhs=masks[i][:], start=st, stop=sp)
                    nc.tensor.matmul(pc1[:],
                                     lhsT=msg_slice(ch, P, 2 * P),
                                     rhs=masks[i][:], start=st, stop=sp)
                if pending is not None:
                    tp, pc0p, pc1p = pending
                    ac = apool.tile([P, FEAT], b16, tag="ac")
                    nc.vector.tensor_copy(out=ac[:, 0:P], in_=pc0p[:])
                    nc.vector.tensor_copy(out=ac[:, P : 2 * P], in_=pc1p[:])
                    tile_tail(1, tp, ac, hdts[tp], None)
                pending = (t, pc0, pc1)
            tp, pc0p, pc1p = pending
            ac = apool.tile([P, FEAT], b16, tag="ac")
            nc.vector.tensor_copy(out=ac[:, 0:P], in_=pc0p[:])
            nc.vector.tensor_copy(out=ac[:, P : 2 * P], in_=pc1p[:])
            tile_tail(1, tp, ac, hdts[tp], None)

            # ================= layer 2: dense sweep over h2res =================
            pc0 = pa.tile([P, P], f32, tag="pc0")
            pc1 = pa.tile([P, P], f32, tag="pc1")
            for j in range(T1):
                st, sp = (j == 0), (j == T1 - 1)
                nc.tensor.matmul(pc0[:], lhsT=h2res[j][:, 0:P],
                                 rhs=sp2_t[:, j * P : (j + 1) * P],
                                 start=st, stop=sp)
                nc.tensor.matmul(pc1[:], lhsT=h2res[j][:, P : 2 * P],
                                 rhs=sp2_t[:, j * P : (j + 1) * P],
                                 start=st, stop=sp)
            # transpose h2res[0] for the self path (identity matmuls)
            ph0 = pa.tile([P, P], f32, tag="pc0")
            ph1 = pa.tile([P, P], f32, tag="pc1")
            nc.tensor.matmul(ph0[:], lhsT=h2res[0][:, 0:P],
                             rhs=ident_t[:], start=True, stop=True)
            nc.tensor.matmul(ph1[:], lhsT=h2res[0][:, P : 2 * P],
                             rhs=ident_t[:], start=True, stop=True)
            hdT2 = apool.tile([P, FEAT], b16, tag="hdT2")
            nc.vector.tensor_copy(out=hdT2[:, 0:P], in_=ph0[:])
            nc.vector.tensor_copy(out=hdT2[:, P : 2 * P], in_=ph1[:])
            ac = apool.tile([P, FEAT], b16, tag="ac")
            nc.vector.tensor_copy(out=ac[:, 0:P], in_=pc0[:])
            nc.vector.tensor_copy(out=ac[:, P : 2 * P], in_=pc1[:])
            tile_tail(2, 0, ac, hdT2, out_d)

    nc.compile()

    in_maps = []
    bf16 = _bf16()
    eye16 = np.eye(P, dtype=bf16)
    for c in range(NCORES):
        m = dict(
            mdov=plan0.mdov[c],
            ident=eye16,
            ones=np.ones((1, P), dtype=bf16),
            gidx1=_wrap_idx16(plan1, c),
            md1=plan1.md[c],
            sp2d=meta["sp2d"][c],
        )
        for t in range(plan0.n_tiles):
            m[f"bandd{t}"] = meta["bandd"][c][t]
            m[f"bandm{t}"] = meta["bandm"][c][t]
        for l in range(3):
            ws, wn, b = meta["weights"][l]
            m[f"ws{l}"] = np.ascontiguousarray(ws.astype(bf16))
            m[f"wn{l}"] = np.ascontiguousarray(wn.astype(bf16))
            m[f"bias{l}"] = np.ascontiguousarray(b[None, :].astype(bf16))
        in_maps.append(m)

    res = run_bass_kernel_spmd(
        nc, in_maps, core_ids=list(range(NCORES)), trace=trace
    )
    if debug_h1:
        return [res.results[c]["out"] for c in range(NCORES)], res, [
            res.results[c]["h1o"] for c in range(NCORES)
        ]
    return [res.results[c]["out"] for c in range(NCORES)], res


def assemble(meta, outs):
    full = np.zeros((NUM_DST[2], OUTW[2]), np.float32)
    for c in range(NCORES):
        full[meta["blocks"][c]["seeds"]] = outs[c]
    return full


def kernel(**inputs) -> np.ndarray:
    meta = build_host(inputs)
    outs, _ = run_device(meta)
    return assemble(meta, outs)


# revision 69
# speedup vs baseline: 1.4573x; 1.4573x over previous
"""DistSAGE 3-layer GraphSAGE forward on 8 TRN2 NeuronCores (Bass/Tile).

Strategy (graph/data parallel, per the DistSAGE recipe):
  - Partition the 512 seed nodes across 8 cores (64 each, LPT-balanced by
    an additive 2-hop cost estimate); build per-core dependency-driven
    blocks on the host (standard DGL block construction, pure index math).
    No inter-core communication; weights replicated.
  - Row-shard the feature table: each core receives compact per-dst-tile
    band tensors (one DRAM tensor per tile -> each load is one fully
    contiguous HBM read; a [128, W] slice of one wide tensor caps at
    ~260 GB/s).  Band tile = [dst rows TRANSPOSED, bf16] + a degree-
    sorted GRID of message rows in fp8e4: chunk k, row d = x[src]/deg of
    the k-th edge of dst d (pre-scaled on host, zero-padded).  Nodes are
    deg-sorted inside each block segment so tiles are degree-homogeneous;
    grid depth is capped at CAP0=11, excess edges spill to a few overflow
    chunks.  fp8 quantization of the (mean-aggregated) messages costs
    ~5e-3 rel err; the self path stays bf16.
  - Aggregation needs NO masks from DRAM and (for grid chunks) no mask
    generation at all: the matmul rhs is the constant identity, so
    aggT[f,d] += msgs_chunk.T @ I accumulates the pre-scaled mean in
    PSUM.  Overflow chunks use a pure one-hot rhs generated on-chip by
    one fused DVE tensor_scalar (colidx == d[p]) from 4 B/slot metadata.
    Each accumulation group owns a full PSUM bank (start=True resets the
    whole bank's has_written -- interleaved groups in one bank corrupt).
  - Y = aggT.T@W_neigh + hdT.T@W_self; dst rows arrive pre-transposed so
    no identity matmuls.  The Y matmuls of tile t-1 are emitted AFTER
    tile t's aggregation matmuls (1-deep software pipeline) so the PE
    never waits on the DVE PSUM->SBUF copy.
  - h1 is stored fp8 in DRAM.  Layer 1 gathers message rows with the
    dma_gather ucode (slot per edge, sources consumer-grouped, NI=512
    calls carrying [lo, hi) row bounds, emitted in readiness order so
    gathers stream under layer-0 compute).  Layer-1 one-hot masks are
    hoisted before the gathers (DVE does them under layer 0); h_dst
    tiles are kept in SBUF in bf16 and transposed by identity matmuls
    (transpose-DMAs on a HWDGE ring stall the whole ring FIFO on their
    h1-write dependency).
  - Layer 2 is a dense mask-matmul sweep over SBUF-resident h2 tiles.
  - DMA rings: Sync carries the band stream (+ final output), Scalar
    carries h1 writes and all consts (weights/metadata upfront; gather
    indices etc. behind the first band tile); gpsimd runs the gathers.
"""

import heapq

import numpy as np

P = 128
NCORES = 8
NUM_DST = (61952, 5632, 512)
FEAT = 256
OUTW = (256, 256, 19)
SEEDS_PER_CORE = NUM_DST[2] // NCORES  # 64
WINDOW = 32768
NI_GATHER = 512  # dma_gather indices per call (layer 1)


def _bf16():
    import ml_dtypes

    return ml_dtypes.bfloat16


def _fp8():
    import ml_dtypes

    return ml_dtypes.float8_e4m3fn


# ---------------------------------------------------------------------------
# Host-side block construction
# ---------------------------------------------------------------------------


def _balance(ids, deg, n_buckets):
    """LPT bin-packing: reorder ids so consecutive 128-groups have ~equal
    total degree (only full 128-groups are balanced)."""
    if n_buckets <= 1 or len(ids) < n_buckets * P:
        return ids
    order = np.argsort(-deg[ids], kind="stable")
    heap = [(0.0, b, 0) for b in range(n_buckets)]
    heapq.heapify(heap)
    buckets = [[] for _ in range(n_buckets)]
    for i in order:
        load, b, cnt = heapq.heappop(heap)
        buckets[b].append(ids[i])
        cnt += 1
        if cnt < P:
            heapq.heappush(heap, (load + deg[ids[i]], b, cnt))
    return np.concatenate([np.asarray(b, dtype=ids.dtype) for b in buckets])


def _seed_partition(esrc0, edst0, esrc1, edst1, esrc2, edst2, deg0, deg1):
    """LPT-balance seeds across cores by an additive 2-hop cost estimate."""
    h = np.zeros(NUM_DST[1], np.float64)
    np.add.at(h, edst1, deg0[esrc1].astype(np.float64))
    cost = np.zeros(NUM_DST[2], np.float64)
    np.add.at(cost, edst2, h[esrc2] + deg1[esrc2].astype(np.float64))
    order = np.argsort(-cost, kind="stable")
    heap = [(0.0, cc, 0) for cc in range(NCORES)]
    heapq.heapify(heap)
    groups = [[] for _ in range(NCORES)]
    for s in order:
        load, cc, cnt = heapq.heappop(heap)
        groups[cc].append(s)
        cnt += 1
        if cnt < SEEDS_PER_CORE:
            heapq.heappush(heap, (load + cost[s], cc, cnt))
    return [np.array(g, dtype=np.int64) for g in groups]


def _block_for_core(seeds, esrc0, edst0, esrc1, edst1, esrc2, edst2,
                    deg0, deg1, deg2):
    # seeds and l1_extra sorted by deg0 (ascending): layer-0 dst tiles
    # become degree-homogeneous, so the grid band layout (chunk k, row d =
    # k-th edge of dst d) pads minimally.
    seeds = seeds[np.argsort(deg0[seeds], kind="stable")]
    pos2 = np.full(NUM_DST[2], -1, np.int32)
    pos2[seeds] = np.arange(SEEDS_PER_CORE, dtype=np.int32)
    sel2 = pos2[edst2] >= 0
    es2, ed2g = esrc2[sel2], edst2[sel2]
    l1_extra = np.setdiff1d(np.unique(es2), seeds)
    l1_extra = l1_extra[np.argsort(deg0[l1_extra], kind="stable")]
    l1_out = np.concatenate([seeds, l1_extra])
    n1 = len(l1_out)

    pos1 = np.full(NUM_DST[1], -1, np.int32)
    pos1[l1_out] = np.arange(n1, dtype=np.int32)
    sel1 = pos1[edst1] >= 0
    es1, ed1g = esrc1[sel1], edst1[sel1]
    ed1 = pos1[ed1g].astype(np.int64)
    inv1 = (1.0 / np.maximum(deg1[ed1g], 1.0)).astype(np.float32)
    # Consumer-grouped l0_extra ordering: [multi-tile srcs | grp0 | grp1 ...]
    # so each layer-1 dst tile's sources sit in a contiguous band of l0 rows
    # -> its gather calls only depend on an early prefix + its own band of
    # h1, enabling overlap of the layer-1 gather under layer-0 compute.
    n1_tiles = -(-n1 // P)
    mask_x = np.ones(len(es1), bool)
    small = es1 < NUM_DST[1]
    mask_x[small] = pos1[es1[small]] < 0
    pr = np.unique(
        np.stack([es1[mask_x], ed1[mask_x] // P], axis=1), axis=0
    )
    srcs_u, first_idx, cnt = np.unique(
        pr[:, 0], return_index=True, return_counts=True
    )
    multi = srcs_u[cnt > 1]
    multi = multi[np.argsort(deg0[multi], kind="stable")]
    segs = [multi]
    single_mask = cnt == 1
    s_srcs = srcs_u[single_mask]
    s_tile = pr[first_idx[single_mask], 1]
    for tt in range(n1_tiles):
        seg = s_srcs[s_tile == tt]
        seg = seg[np.argsort(deg0[seg], kind="stable")]
        segs.append(seg)

    ed2 = pos2[ed2g].astype(np.int64)
    inv2 = (1.0 / np.maximum(deg2[ed2g], 1.0)).astype(np.float32)
    es2l = pos1[es2].astype(np.int64)

    return dict(
        seeds=seeds, l1_out=l1_out, segs=segs, n1=n1,
        e1g=(es1, ed1, inv1),
        e2=(es2l, ed2, inv2),
    )


def _assemble_l0(blocks, esrc0, edst0, deg0):
    """Build per-core l0_out = [l1_out | segs...] (tight packing), padded
    to a common n0_pad with duplicates of row 0 (they carry no edges).
    Fills blocks[c]['l0_out'], 'n0', 'e0' (local), 'e1' (local srcs)."""
    n1_pad = max(-(-b["n1"] // P) for b in blocks) * P
    n0_pad = -(-max(
        b["n1"] + sum(len(s) for s in b["segs"]) for b in blocks
    ) // P) * P
    for b in blocks:
        cat = np.concatenate([b["l1_out"]] + b["segs"])
        l0 = np.zeros(n0_pad, np.int64)
        l0[: len(cat)] = cat
        l0[len(cat) :] = cat[0]
        pos0 = np.full(NUM_DST[0], -1, np.int32)
        pos0[cat] = np.arange(len(cat), dtype=np.int32)
        b["l0_out"] = l0
        b["n0"] = n0_pad

        sel0 = pos0[edst0] >= 0
        es0, ed0g = esrc0[sel0], edst0[sel0]
        ed0 = pos0[ed0g].astype(np.int64)
        inv0 = (1.0 / np.maximum(deg0[ed0g], 1.0)).astype(np.float32)
        b["e0"] = (es0.astype(np.int64), ed0, inv0)
        es1, ed1, inv1 = b["e1g"]
        b["e1"] = (pos0[es1].astype(np.int64), ed1, inv1)
    return n0_pad, n1_pad


def _slots_by_tile(es, ed, inv, n_tiles):
    """Slot-per-edge: per dst tile, edge slots sorted by src row.
    Returns per-tile (srcs, dloc, inv) arrays (dloc = dst index in tile)."""
    tile = ed // P
    order = np.lexsort((es, tile))
    es, ed, inv, tile = es[order], ed[order], inv[order], tile[order]
    starts = np.searchsorted(tile, np.arange(n_tiles))
    ends = np.searchsorted(tile, np.arange(n_tiles) + 1)
    return [
        (es[s:e], (ed[s:e] - t * P).astype(np.int64), inv[s:e])
        for t, (s, e) in enumerate(zip(starts, ends))
    ]


CAP0 = 11  # grid chunks per tile cap; excess edges go to overflow chunks


def _grid_by_tile(es, ed, inv, n_tiles, cap):
    """Grid layout: per dst tile, edge slot = rank*128 + dloc (rank = edge
    index within its dst), capped at `cap` ranks; excess edges spill to an
    overflow list (slot-per-edge with one-hot masks)."""
    tile = ed // P
    order = np.lexsort((es, ed))
    es, ed, inv = es[order], ed[order], inv[order]
    tile = tile[order]
    starts = np.searchsorted(tile, np.arange(n_tiles))
    ends = np.searchsorted(tile, np.arange(n_tiles) + 1)
    out = []
    for t, (s, e) in enumerate(zip(starts, ends)):
        dloc = (ed[s:e] - t * P).astype(np.int64)
        cnt = np.bincount(dloc, minlength=P)
        first = np.concatenate([[0], np.cumsum(cnt)[:-1]])
        rank = np.arange(e - s) - first[dloc]  # edges sorted by dloc
        ing = rank < cap
        out.append(dict(
            gsrc=es[s:e][ing], gslot=rank[ing] * P + dloc[ing],
            ginv=inv[s:e][ing],
            K=int(min(cnt.max(), cap)) if e > s else 0,
            osrc=es[s:e][~ing], od=dloc[~ing], oinv=inv[s:e][~ing],
        ))
    return out


class GatherPlan:
    """Layer 1: continuous slot stream gathered via dma_gather (one slot
    per edge).  Tile t owns stream slots [slot_off[t], slot_off[t]+m[t]);
    chunks are 128-slot groups; a chunk overlapping two tiles gets one
    metadata column per tile.  Calls are tile-aligned (big calls + a small
    tail call per tile) and carry [lo, hi) row bounds so each call only
    depends on the h-table rows it reads."""

    def __init__(self, n_tiles, slot_counts, ni):
        self.ni = ni
        self.cpc = ni // P  # max chunks per big call
        self.n_tiles = n_tiles
        self.m = slot_counts
        self.slot_off = np.concatenate([[0], np.cumsum(slot_counts)]).astype(np.int64)
        total = int(self.slot_off[-1])
        self.n_chunks = -(-total // P)
        self.n_chunks_pad = self.n_chunks
        # tile-aligned call partition: big calls + small tail call per tile
        TAILC = 2
        self.call_sizes = []
        for t in range(n_tiles):
            s = -(-int(self.slot_off[t]) // P)
            e = -(-int(self.slot_off[t + 1]) // P) if t + 1 < n_tiles else self.n_chunks
            if t + 1 == n_tiles:
                e = self.n_chunks
            nch = e - s
            if nch <= 0:
                continue
            if nch > TAILC + 1:
                head = nch - TAILC
                nbig = -(-head // self.cpc)
                base = head // nbig
                rem = head - base * nbig
                self.call_sizes += [base + (1 if i < rem else 0)
                                    for i in range(nbig)]
                self.call_sizes.append(TAILC)
            else:
                self.call_sizes.append(nch)
        assert sum(self.call_sizes) == self.n_chunks
        self.n_calls = len(self.call_sizes)
        self.call_chunk_off = np.concatenate(
            [[0], np.cumsum(self.call_sizes)]
        ).astype(np.int64)
        self.pairs = []
        self.tile_pairs = []  # per tile: list of (sp_col, chunk)
        for t in range(n_tiles):
            lo, hi = int(self.slot_off[t]), int(self.slot_off[t + 1])
            ch1 = (hi - 1) // P if hi > lo else lo // P
            tp = []
            for ch in range(lo // P, ch1 + 1):
                tp.append((len(self.pairs), ch))
                self.pairs.append((t, ch))
            self.tile_pairs.append(tp)
        self.n_sp_cols = len(self.pairs)
        self.gidx = []  # [NCORES][128, n_chunks_pad] int64 table rows
        self.md = []  # [NCORES][128, n_sp_cols*2] f32 (d, inv) per slot
        self.call_base = None  # [n_calls] row base (lo) per call
        self.call_hi = None  # [n_calls] exclusive row bound per call

    def compute_call_bounds(self, nrows):
        """Per-call [lo, hi) over all cores, 128-aligned."""
        lo = np.zeros(self.n_calls, np.int64)
        hi = np.zeros(self.n_calls, np.int64)
        for k in range(self.n_calls):
            a, b = int(self.call_chunk_off[k]), int(self.call_chunk_off[k + 1])
            mn, mx = nrows, 0
            for g in self.gidx:
                sl = g[:, a:b]
                mn = min(mn, int(sl.min()))
                mx = max(mx, int(sl.max()))
            lo[k] = (mn // P) * P
            hi[k] = min(nrows, ((mx // P) + 1) * P)
        self.call_base = lo
        self.call_hi = hi


def _fill_gather(plan, per_core_tiles, pad_row):
    total_pad = plan.n_chunks_pad * P
    for c in range(NCORES):
        stream = np.zeros(total_pad, np.int64)
        dstream = np.full(total_pad, -1.0, np.float32)
        istream = np.zeros(total_pad, np.float32)
        for t in range(plan.n_tiles):
            lo, hi = int(plan.slot_off[t]), int(plan.slot_off[t + 1])
            srcs, dloc, inv = per_core_tiles[c][t]
            stream[lo : lo + len(srcs)] = srcs
            stream[lo + len(srcs) : hi] = pad_row[c][t]
            dstream[lo : lo + len(srcs)] = dloc
            istream[lo : lo + len(srcs)] = inv
        tail = int(plan.slot_off[-1])
        stream[tail:] = pad_row[c][plan.n_tiles - 1]
        plan.gidx.append(stream.reshape(plan.n_chunks_pad, P).T.copy())

        # metadata: per (tile, chunk) pair one (d, inv) column; slots of
        # the chunk outside the tile's range get (-1, 0).
        md = np.zeros((P, plan.n_sp_cols, 2), np.float32)
        md[:, :, 0] = -1.0
        for t in range(plan.n_tiles):
            lo, hi = int(plan.slot_off[t]), int(plan.slot_off[t + 1])
            for sp_col, ch in plan.tile_pairs[t]:
                s0 = ch * P
                a = max(s0, lo)
                b = min(s0 + P, hi)
                if a < b:
                    md[a - s0 : b - s0, sp_col, 0] = dstream[a:b]
                    md[a - s0 : b - s0, sp_col, 1] = istream[a:b]
        plan.md.append(np.ascontiguousarray(md.reshape(P, plan.n_sp_cols * 2)))


class BandPlan:
    """Layer 0: per-tile dense bands, pre-interleaved.  Tile t's band =
    group 0 (dst rows TRANSPOSED: [f0..127 x d | f128..255 x d]) +
    K[t] grid groups (chunk k row d = x[src]/deg of the k-th edge of dst
    d, pre-scaled on host, zero-padded -> aggregation rhs is the constant
    identity) + O[t] overflow groups (slot-per-edge, pre-scaled, one-hot
    is_equal masks from per-slot dst metadata)."""

    def __init__(self, n_tiles, K, O):
        self.n_tiles = n_tiles
        self.K = [max(1, k) for k in K]  # capped max-over-core degree
        self.O = O  # overflow chunks per tile (max over cores)
        self.goff = np.concatenate(
            [[0], np.cumsum([1 + k + o for k, o in zip(self.K, O)])]
        ).astype(np.int64)
        self.n_groups = int(self.goff[-1])
        self.ov_off = np.concatenate([[0], np.cumsum(O)]).astype(np.int64)
        self.n_ov_cols = int(self.ov_off[-1])
        self.mdov = []  # [NCORES][128, n_ov_cols] f32 dst-id per slot


def build_host(inputs):
    esrc0 = np.asarray(inputs["esrc0"]).astype(np.int64)
    edst0 = np.asarray(inputs["edst0"]).astype(np.int64)
    esrc1 = np.asarray(inputs["esrc1"]).astype(np.int64)
    edst1 = np.asarray(inputs["edst1"]).astype(np.int64)
    esrc2 = np.asarray(inputs["esrc2"]).astype(np.int64)
    edst2 = np.asarray(inputs["edst2"]).astype(np.int64)
    x = np.asarray(inputs["x"], dtype=np.float32)

    deg0 = np.bincount(edst0, minlength=NUM_DST[0]).astype(np.float32)
    deg1 = np.bincount(edst1, minlength=NUM_DST[1]).astype(np.float32)
    deg2 = np.bincount(edst2, minlength=NUM_DST[2]).astype(np.float32)

    seed_groups = _seed_partition(esrc0, edst0, esrc1, edst1, esrc2, edst2,
                                  deg0, deg1)
    blocks = [
        _block_for_core(seed_groups[c], esrc0, edst0, esrc1, edst1, esrc2,
                        edst2, deg0, deg1, deg2)
        for c in range(NCORES)
    ]
    n0_pad, n1_pad = _assemble_l0(blocks, esrc0, edst0, deg0)
    T0, T1, T2 = n0_pad // P, n1_pad // P, 1

    tiles0 = [_grid_by_tile(*b["e0"], T0, CAP0) for b in blocks]
    tiles1 = [_slots_by_tile(*b["e1"], T1) for b in blocks]

    # ---- layer 0: band plan + pre-interleaved compact tables ----
    plan0 = BandPlan(
        T0,
        [max(tiles0[c][t]["K"] for c in range(NCORES)) for t in range(T0)],
        [max(-(-len(tiles0[c][t]["osrc"]) // P) for c in range(NCORES))
         for t in range(T0)],
    )
    l0_padded = [b["l0_out"] for b in blocks]

    bf16 = _bf16()
    fp8 = _fp8()
    x16 = x.astype(bf16)
    # per-core, per-tile: dst groups (bf16, transposed) + msg grids (fp8)
    bandd, bandm = [], []
    for c in range(NCORES):
        bd, bm = [], []
        mdov = np.full((P, max(plan0.n_ov_cols, 1)), -1.0, np.float32)
        for t in range(T0):
            hd = x16[l0_padded[c][t * P : (t + 1) * P]]  # [128 dst, 256 f]
            bd.append(np.ascontiguousarray(
                hd.T.reshape(2, P, P).transpose(1, 0, 2).reshape(P, FEAT)
            ))
            # bd[t][p, h*128+j] = hd[j, h*128+p]
            ti = tiles0[c][t]
            K, O = plan0.K[t], plan0.O[t]
            grid = np.zeros(((K + O) * P, FEAT), np.float32)
            grid[ti["gslot"]] = x[ti["gsrc"]] * ti["ginv"][:, None]
            if O:
                no = len(ti["osrc"])
                grid[K * P : K * P + no] = x[ti["osrc"]] * ti["oinv"][:, None]
                oo = int(plan0.ov_off[t])
                dcol = np.full(O * P, -1.0, np.float32)
                dcol[:no] = ti["od"]
                mdov[:, oo : oo + O] = dcol.reshape(O, P).T
            g8 = grid.astype(fp8).reshape(K + O, P, FEAT)
            bm.append(np.ascontiguousarray(
                g8.transpose(1, 0, 2).reshape(P, (K + O) * FEAT)
            ))
        bandd.append(bd)
        bandm.append(bm)
        plan0.mdov.append(np.ascontiguousarray(mdov))

    # ---- layer 1: gather plan (slot per edge) ----
    m1 = [
        max(1, max(len(tiles1[c][t][0]) for c in range(NCORES)))
        for t in range(T1)
    ]
    plan1 = GatherPlan(T1, m1, NI_GATHER)
    padL = [[t * P for t in range(T1)] for _ in range(NCORES)]
    _fill_gather(plan1, tiles1, padL)
    plan1.compute_call_bounds(n0_pad)
    assert n0_pad <= WINDOW and n1_pad <= WINDOW

    # ---- layer 2: dense sweep over SBUF-resident h2 (no gather) ----
    # sp2dense[c][j] = [128, 128] mask: W[row, seed] = sum inv2 over edges
    # (src local j*128+row -> seed).
    sp2d = []
    for c in range(NCORES):
        es, ed, inv = blocks[c]["e2"]
        W = np.zeros((T1, P, P), np.float32)
        np.add.at(W, (es // P, es % P, ed), inv)
        sp2d.append(
            np.ascontiguousarray(
                W.transpose(1, 0, 2).reshape(P, T1 * P).astype(bf16)
            )
        )

    return dict(
        plan0=plan0,
        plans=(plan1,),
        sp2d=sp2d,
        T=(T0, T1, T2),
        n0_pad=n0_pad,
        n1_pad=n1_pad,
        bandd=bandd,
        bandm=bandm,
        blocks=blocks,
        weights=tuple(
            (
                np.asarray(inputs[f"W_self{l}"], np.float32),
                np.asarray(inputs[f"W_neigh{l}"], np.float32),
                np.asarray(inputs[f"b{l}"], np.float32),
            )
            for l in range(3)
        ),
    )


# ---------------------------------------------------------------------------
# Numpy simulation of the device kernel (validation aid; fp32 stand-in)
# ---------------------------------------------------------------------------


def simulate_core(meta, c):
    plan0 = meta["plan0"]
    colidx = np.arange(P, dtype=np.float32)

    mdov = plan0.mdov[c]
    ws, wn, b = meta["weights"][0]
    table = np.zeros((plan0.n_tiles * P, OUTW[0]), np.float32)
    for t in range(plan0.n_tiles):
        K, O = plan0.K[t], plan0.O[t]
        hdT = meta["bandd"][c][t].astype(np.float32)
        hd = np.concatenate([hdT[:, 0:P].T, hdT[:, P : 2 * P].T], axis=1)
        xm = meta["bandm"][c][t].astype(np.float32).reshape(P, K + O, FEAT)
        mean = xm[:, 0:K, :].sum(axis=1)  # [d, f]
        oo = int(plan0.ov_off[t])
        for o in range(O):
            msgs = xm[:, K + o, :]
            onehot = colidx[None, :] == mdov[:, oo + o : oo + o + 1]
            mean += (msgs.T @ onehot).T
        table[t * P : (t + 1) * P] = np.maximum(hd @ ws + mean @ wn + b, 0.0)

    plan = meta["plans"][0]
    md1 = plan.md[c].reshape(P, plan.n_sp_cols, 2)
    table8 = table.astype(_fp8()).astype(np.float32)  # h1buf is fp8
    ws, wn, b = meta["weights"][1]
    out = np.zeros((plan.n_tiles * P, OUTW[1]), np.float32)
    for t in range(plan.n_tiles):
        hd = table[t * P : (t + 1) * P]
        aggT = np.zeros((FEAT, P), np.float32)
        for sp_col, ch in plan.tile_pairs[t]:
            msgs = table8[plan.gidx[c][:, ch]]
            mask = (colidx[None, :] == md1[:, sp_col, 0:1]) * md1[:, sp_col, 1:2]
            aggT += msgs.T @ mask
        out[t * P : (t + 1) * P] = np.maximum(hd @ ws + aggT.T @ wn + b, 0.0)
    table = out

    # layer 2: dense sweep
    ws, wn, b = meta["weights"][2]
    sp2 = meta["sp2d"][c].astype(np.float32).reshape(P, -1, P)
    hd = table[0:P]
    aggT = np.zeros((FEAT, P), np.float32)
    for j in range(sp2.shape[1]):
        aggT += table[j * P : (j + 1) * P].T @ sp2[:, j, :]
    y = hd @ ws + aggT.T @ wn + b
    return y[:SEEDS_PER_CORE]


# ---------------------------------------------------------------------------
# Device kernel
# ---------------------------------------------------------------------------


def _wrap_idx16(plan, c):
    bases = np.zeros(plan.n_chunks_pad, np.int64)
    for k in range(plan.n_calls):
        bases[plan.call_chunk_off[k] : plan.call_chunk_off[k + 1]] = plan.call_base[k]
    rel = plan.gidx[c] - bases[None, :]
    total16 = plan.n_chunks_pad * P // 16
    out = np.zeros((P, total16), np.int16)
    off16 = 0
    for k in range(plan.n_calls):
        a, b = int(plan.call_chunk_off[k]), int(plan.call_chunk_off[k + 1])
        flat = rel[:, a:b].T.reshape(-1)
        w = flat.reshape(len(flat) // 16, 16).T.astype(np.int16)
        out[:16, off16 : off16 + w.shape[1]] = w
        off16 += w.shape[1]
    for rep in range(1, 8):
        out[rep * 16 : (rep + 1) * 16] = out[:16]
    return out


def run_device(meta, trace=False, debug_h1=False):
    import concourse.bacc as bacc
    import concourse.tile as tile
    import concourse.mybir as mybir
    from concourse.bass_utils import run_bass_kernel_spmd

    plan0 = meta["plan0"]
    plan1 = meta["plans"][0]
    T1 = meta["T"][1]
    f32 = mybir.dt.float32
    i32 = mybir.dt.int32
    b16 = mybir.dt.bfloat16
    f8 = mybir.dt.float8e4
    alu = mybir.AluOpType

    nc = bacc.Bacc("TRN2", target_bir_lowering=False, debug=False, num_devices=NCORES)

    # one DRAM tensor per band tile: the transfer is then one fully
    # contiguous HBM region (a [P, W] slice of a wide tensor reads 128
    # scattered ~7KB segments and caps at ~260 GB/s).  dst groups bf16,
    # message grids fp8 (mean-aggregated -> quantization error washes out)
    bandd_d = [
        nc.dram_tensor(f"bandd{t}", [P, FEAT], b16, kind="ExternalInput")
        for t in range(plan0.n_tiles)
    ]
    bandm_d = [
        nc.dram_tensor(
            f"bandm{t}",
            [P, (plan0.K[t] + plan0.O[t]) * FEAT],
            f8,
            kind="ExternalInput",
        )
        for t in range(plan0.n_tiles)
    ]
    mdov_d = nc.dram_tensor("mdov", [P, max(plan0.n_ov_cols, 1)], f32,
                            kind="ExternalInput")
    ident_d = nc.dram_tensor("ident", [P, P], b16, kind="ExternalInput")
    ones_d = nc.dram_tensor("ones", [1, P], b16, kind="ExternalInput")
    h1buf = nc.dram_tensor("h1buf", [meta["n0_pad"], FEAT], f8)
    out_d = nc.dram_tensor("out", [SEEDS_PER_CORE, OUTW[2]], f32, kind="ExternalOutput")

    h1o_d = None
    if debug_h1:
        h1o_d = nc.dram_tensor("h1o", [meta["n0_pad"], FEAT], f32,
                               kind="ExternalOutput")
    idx1_d = nc.dram_tensor("gidx1", [P, plan1.n_chunks_pad * P // 16],
                            mybir.dt.int16, kind="ExternalInput")
    md1_d = nc.dram_tensor("md1", [P, plan1.n_sp_cols * 2], f32,
                           kind="ExternalInput")
    sp2_d = nc.dram_tensor("sp2d", [P, T1 * P], b16, kind="ExternalInput")
    w_d = []
    for l in range(3):
        w_d.append(
            (
                nc.dram_tensor(f"ws{l}", [FEAT, OUTW[l]], b16, kind="ExternalInput"),
                nc.dram_tensor(f"wn{l}", [FEAT, OUTW[l]], b16, kind="ExternalInput"),
                nc.dram_tensor(f"bias{l}", [1, OUTW[l]], b16, kind="ExternalInput"),
            )
        )

    use_bias = [bool(np.any(meta["weights"][l][2] != 0)) for l in range(3)]

    with tile.TileContext(nc) as tc:
        with (
            tc.tile_pool(name="const", bufs=1) as cpool,
            tc.tile_pool(name="msgs", bufs=12) as mpool,
            tc.tile_pool(name="dsts", bufs=12) as dpool,
            tc.tile_pool(name="mask", bufs=28) as kpool,
            tc.tile_pool(name="acc", bufs=3) as apool,
            tc.tile_pool(name="outp", bufs=3) as opool,
            tc.tile_pool(name="hdt", bufs=3) as hpool,
            tc.tile_pool(name="gmsg", bufs=1) as gpool,
            tc.tile_pool(name="pagg", bufs=2, space="PSUM") as pa,
            tc.tile_pool(name="py", bufs=2, space="PSUM") as pypool,
        ):
            # ---- upfront consts ----
            ident_t = cpool.tile([P, P], b16, tag="ident")
            nc.scalar.dma_start(out=ident_t[:], in_=ident_d[:])
            mdov_t = cpool.tile([P, max(plan0.n_ov_cols, 1)], f32, tag="mdov")
            nc.scalar.dma_start(out=mdov_t[:], in_=mdov_d[:])
            md1_t = cpool.tile([P, plan1.n_sp_cols * 2], f32, tag="md1")
            nc.scalar.dma_start(out=md1_t[:], in_=md1_d[:])
            colidx_i = cpool.tile([P, P], i32, tag="colidx_i")
            nc.gpsimd.iota(colidx_i[:], [[1, P]], channel_multiplier=0)
            colidx = cpool.tile([P, P], f32, tag="colidx")
            nc.vector.tensor_copy(out=colidx[:], in_=colidx_i[:])

            ws_ts, wn_ts, bias_ts = [[None, None] for _ in range(3)], \
                [[None, None] for _ in range(3)], [None] * 3
            ones_t = cpool.tile([1, P], b16, tag="ones")

            def load_weights(l, eng):
                outw = OUTW[l]
                for k in range(2):
                    w = cpool.tile([P, outw], b16, tag=f"ws{l}_{k}")
                    eng.dma_start(out=w[:], in_=w_d[l][0][k * P : (k + 1) * P, :])
                    ws_ts[l][k] = w
                    w = cpool.tile([P, outw], b16, tag=f"wn{l}_{k}")
                    eng.dma_start(out=w[:], in_=w_d[l][1][k * P : (k + 1) * P, :])
                    wn_ts[l][k] = w
                if use_bias[l]:
                    bias_t = cpool.tile([1, outw], b16, tag=f"bias{l}")
                    eng.dma_start(out=bias_t[:], in_=w_d[l][2][:])
                    bias_ts[l] = bias_t

            load_weights(0, nc.scalar)
            if any(use_bias):
                nc.scalar.dma_start(out=ones_t[:], in_=ones_d[:])

            h2res = [
                cpool.tile([P, FEAT], b16, tag=f"h2res_{t}", name=f"h2res_{t}")
                for t in range(T1)
            ]
            h1res = [
                cpool.tile([P, FEAT], b16, tag=f"h1res_{t}", name=f"h1res_{t}")
                for t in range(T1)
            ]

            def gen_mask(md_t, col):
                """One-hot mask [128 slots, 128 dst] = (colidx==d[p]) * inv[p]."""
                mk = kpool.tile([P, P], b16, tag="mk")
                nc.vector.tensor_scalar(
                    out=mk[:],
                    in0=colidx[:],
                    scalar1=md_t[:, 2 * col : 2 * col + 1],
                    scalar2=md_t[:, 2 * col + 1 : 2 * col + 2],
                    op0=alu.is_equal,
                    op1=alu.mult,
                )
                return mk

            def gen_mask_ov(col):
                """Pure one-hot [128 slots, 128 dst] = (colidx==d[p])."""
                mk = kpool.tile([P, P], b16, tag="mk")
                nc.vector.tensor_scalar(
                    out=mk[:],
                    in0=colidx[:],
                    scalar1=mdov_t[:, col : col + 1],
                    scalar2=None,
                    op0=alu.is_equal,
                )
                return mk

            def tile_tail(l, t, ac, hdT, dest):
                """Y matmuls + bias + activation + store for one dst tile.
                ac = aggT halves [f-half, d] bf16; hdT = dst rows transposed."""
                outw = OUTW[l]
                y = pypool.tile([P, outw], f32, tag="y")
                nc.tensor.matmul(y[:], lhsT=ac[:, 0:P], rhs=wn_ts[l][0][:],
                                 start=True, stop=False)
                nc.tensor.matmul(y[:], lhsT=ac[:, P : 2 * P], rhs=wn_ts[l][1][:],
                                 start=False, stop=False)
                nc.tensor.matmul(y[:], lhsT=hdT[:, 0:P], rhs=ws_ts[l][0][:],
                                 start=False, stop=False)
                nc.tensor.matmul(y[:], lhsT=hdT[:, P : 2 * P], rhs=ws_ts[l][1][:],
                                 start=False, stop=not use_bias[l])
                if use_bias[l]:
                    nc.tensor.matmul(y[:], lhsT=ones_t[0:1, :],
                                     rhs=bias_ts[l][0:1, :],
                                     start=False, stop=True)
                if l == 0:
                    o2 = opool.tile([P, outw], f8, tag="o2")
                    nc.scalar.activation(
                        out=o2[:], in_=y[:],
                        func=mybir.ActivationFunctionType.Relu,
                    )
                    nc.scalar.dma_start(out=dest[t * P : (t + 1) * P, :], in_=o2[:])
                    if t < T1:
                        # bf16 copy kept on-chip for the layer-1 self path
                        nc.scalar.activation(
                            out=h1res[t][:], in_=y[:],
                            func=mybir.ActivationFunctionType.Relu,
                        )
                    if debug_h1:
                        od = opool.tile([P, outw], f32, tag="od")
                        nc.vector.tensor_copy(out=od[:], in_=o2[:])
                        nc.sync.dma_start(
                            out=h1o_d[t * P : (t + 1) * P, :], in_=od[:]
                        )
                elif l == 1:
                    nc.scalar.activation(
                        out=h2res[t][:], in_=y[:],
                        func=mybir.ActivationFunctionType.Relu,
                    )
                else:
                    o = opool.tile([P, outw], f32, tag="o")
                    nc.vector.tensor_copy(out=o[:], in_=y[:])
                    nc.sync.dma_start(out=dest[:], in_=o[0:SEEDS_PER_CORE, :])

            # ================= layer 0: dense bands =================
            # 1-deep software pipeline: tile t's mask matmuls are emitted
            # before tile t-1's PSUM copy + Y matmuls, so the PE never
            # waits on the DVE copy.
            Kmax = max(k + o for k, o in zip(plan0.K, plan0.O))
            pending = None  # (t, pc0, pc1, hdT_view)
            for t in range(plan0.n_tiles):
                K, O = plan0.K[t], plan0.O[t]
                oo = int(plan0.ov_off[t])
                btd = dpool.tile([P, FEAT], b16, tag="bandd")
                nc.sync.dma_start(out=btd[:], in_=bandd_d[t][:])
                btm = mpool.tile([P, Kmax * FEAT], f8, tag="bandm")
                nc.sync.dma_start(
                    out=btm[:, : (K + O) * FEAT], in_=bandm_d[t][:]
                )
                omasks = [gen_mask_ov(oo + o) for o in range(O)]
                # two PSUM tiles: each accumulation group must own its bank
                # (start=True resets the whole bank's has_written); rhs is
                # the constant identity for grid chunks (rows pre-scaled,
                # dst-aligned), a one-hot mask for overflow chunks
                pc0 = pa.tile([P, P], f32, tag="pc0")
                pc1 = pa.tile([P, P], f32, tag="pc1")
                for k in range(K + O):
                    st, sp = (k == 0), (k == K + O - 1)
                    base = k * FEAT
                    rhs = ident_t[:] if k < K else omasks[k - K][:]
                    nc.tensor.matmul(pc0[:], lhsT=btm[:, base : base + P],
                                     rhs=rhs, start=st, stop=sp)
                    nc.tensor.matmul(pc1[:],
                                     lhsT=btm[:, base + P : base + 2 * P],
                                     rhs=rhs, start=st, stop=sp)
                if pending is not None:
                    tp, pc0p, pc1p, hdTp = pending
                    ac = apool.tile([P, FEAT], b16, tag="ac")
                    nc.vector.tensor_copy(out=ac[:, 0:P], in_=pc0p[:])
                    nc.vector.tensor_copy(out=ac[:, P : 2 * P], in_=pc1p[:])
                    tile_tail(0, tp, ac, hdTp, h1buf)
                pending = (t, pc0, pc1, btd[:])

                if t == 0:
                    # late consts: emitted behind the first band loads so the
                    # main stream starts immediately; all are ready long
                    # before their consumers run.
                    idx1_t = cpool.tile(list(idx1_d.shape), mybir.dt.int16,
                                        tag="idx1")
                    nc.scalar.dma_start(out=idx1_t[:], in_=idx1_d[:])
                    load_weights(1, nc.scalar)
                    load_weights(2, nc.scalar)
                    sp2_t = cpool.tile([P, T1 * P], b16, tag="sp2d")
                    nc.scalar.dma_start(out=sp2_t[:], in_=sp2_d[:])
            # flush the pipeline
            tp, pc0p, pc1p, hdTp = pending
            ac = apool.tile([P, FEAT], b16, tag="ac")
            nc.vector.tensor_copy(out=ac[:, 0:P], in_=pc0p[:])
            nc.vector.tensor_copy(out=ac[:, P : 2 * P], in_=pc1p[:])
            tile_tail(0, tp, ac, hdTp, h1buf)

            # ================= layer 1: overlapped gather =================
            # hoist all layer-1 mask generation (depends only on md1/colidx)
            # so the DVE does it under layer-0 compute instead of in the
            # gather-bound tail (NOT on gpsimd: Q7 tensor_scalar is ~20x
            # slower and serializes ahead of the gather descriptor gen)
            l1_masks = []
            for t in range(plan1.n_tiles):
                tm = []
                for i, (sp_col, _) in enumerate(plan1.tile_pairs[t]):
                    mk = cpool.tile([P, P], b16, tag=f"mk1_{t}_{i}",
                                    name=f"mk1_{t}_{i}")
                    nc.vector.tensor_scalar(
                        out=mk[:],
                        in0=colidx[:],
                        scalar1=md1_t[:, 2 * sp_col : 2 * sp_col + 1],
                        scalar2=md1_t[:, 2 * sp_col + 1 : 2 * sp_col + 2],
                        op0=alu.is_equal,
                        op1=alu.mult,
                    )
                    tm.append(mk)
                l1_masks.append(tm)

            # transposed h_dst tiles via identity matmuls from the
            # SBUF-resident h1res copies (no DMA-ring blocking)
            hdts = []
            for t in range(T1):
                ph = pa.tile([P, FEAT], f32, tag="pht", name=f"pht_{t}")
                nc.tensor.matmul(ph[:, 0:P], lhsT=h1res[t][:, 0:P],
                                 rhs=ident_t[:], start=True, stop=True)
                nc.tensor.matmul(ph[:, P : 2 * P], lhsT=h1res[t][:, P : 2 * P],
                                 rhs=ident_t[:], start=True, stop=True)
                ht = hpool.tile([P, FEAT], b16, tag=f"hdt_{t}", name=f"hdt_{t}")
                nc.vector.tensor_copy(out=ht[:], in_=ph[:])
                hdts.append(ht)

            call_tiles = [None] * plan1.n_calls
            order = sorted(
                range(plan1.n_calls),
                key=lambda k: (int(plan1.call_hi[k]), int(plan1.call_base[k])),
            )
            for k in order:
                a = int(plan1.call_chunk_off[k])
                b2 = int(plan1.call_chunk_off[k + 1])
                sz = b2 - a
                lo = int(plan1.call_base[k])
                hi = int(plan1.call_hi[k])
                mt = gpool.tile([P, sz * FEAT], f8, tag=f"msgs1_{k}")
                nc.gpsimd.dma_gather(
                    out_ap=mt[:, : sz * FEAT].rearrange("p (g d) -> p g d", g=sz),
                    in_ap=h1buf[lo:hi, :],
                    idxs_ap=idx1_t[:, a * P // 16 : b2 * P // 16],
                    num_idxs=sz * P,
                    num_idxs_reg=sz * P,
                    elem_size=FEAT,
                    single_packet=False,
                )
                call_tiles[k] = (mt, a)

            call_of_chunk = np.searchsorted(
                plan1.call_chunk_off, np.arange(plan1.n_chunks_pad), side="right"
            ) - 1

            def msg_slice(ch, f0, f1):
                k = int(call_of_chunk[ch])
                mt, a = call_tiles[k]
                j = ch - a
                return mt[:, j * FEAT + f0 : j * FEAT + f1]

            pending = None
            for t in range(plan1.n_tiles):
                pairs = plan1.tile_pairs[t]
                masks = l1_masks[t]
                pc0 = pa.tile([P, P], f32, tag="pc0")
                pc1 = pa.tile([P, P], f32, tag="pc1")
                for i, (sp_col, ch) in enumerate(pairs):
                    st, sp = (i == 0), (i == len(pairs) - 1)
                    nc.tensor.matmul(pc0[:], lhsT=msg_slice(ch, 0, P),
                                     rhs=masks[i][:], start=st, stop=sp)
                    nc.tensor.matmul(pc1[:],
                                     lhsT=msg_slice(ch, P, 2 * P),
                                     rhs=masks[i][:], start=st, stop=sp)
                if pending is not None:
                    tp, pc0p, pc1p = pending
                    ac = apool.tile([P, FEAT], b16, tag="ac")
                    nc.vector.tensor_copy(out=ac[:, 0:P], in_=pc0p[:])
                    nc.vector.tensor_copy(out=ac[:, P : 2 * P], in_=pc1p[:])
                    tile_tail(1, tp, ac, hdts[tp], None)
                pending = (t, pc0, pc1)
            tp, pc0p, pc1p = pending
            ac = apool.tile([P, FEAT], b16, tag="ac")
            nc.vector.tensor_copy(out=ac[:, 0:P], in_=pc0p[:])
            nc.vector.tensor_copy(out=ac[:, P : 2 * P], in_=pc1p[:])
            tile_tail(1, tp, ac, hdts[tp], None)

            # ================= layer 2: dense sweep over h2res =================
            pc0 = pa.tile([P, P], f32, tag="pc0")
            pc1 = pa.tile([P, P], f32, tag="pc1")
            for j in range(T1):
                st, sp = (j == 0), (j == T1 - 1)
                nc.tensor.matmul(pc0[:], lhsT=h2res[j][:, 0:P],
                                 rhs=sp2_t[:, j * P : (j + 1) * P],
                                 start=st, stop=sp)
                nc.tensor.matmul(pc1[:], lhsT=h2res[j][:, P : 2 * P],
                                 rhs=sp2_t[:, j * P : (j + 1) * P],
                                 start=st, stop=sp)
            # transpose h2res[0] for the self path (identity matmuls)
            ph0 = pa.tile([P, P], f32, tag="pc0")
            ph1 = pa.tile([P, P], f32, tag="pc1")
            nc.tensor.matmul(ph0[:], lhsT=h2res[0][:, 0:P],
                             rhs=ident_t[:], start=True, stop=True)
            nc.tensor.matmul(ph1[:], lhsT=h2res[0][:, P : 2 * P],
                             rhs=ident_t[:], start=True, stop=True)
            hdT2 = apool.tile([P, FEAT], b16, tag="hdT2")
            nc.vector.tensor_copy(out=hdT2[:, 0:P], in_=ph0[:])
            nc.vector.tensor_copy(out=hdT2[:, P : 2 * P], in_=ph1[:])
            ac = apool.tile([P, FEAT], b16, tag="ac")
            nc.vector.tensor_copy(out=ac[:, 0:P], in_=pc0[:])
            nc.vector.tensor_copy(out=ac[:, P : 2 * P], in_=pc1[:])
            tile_tail(2, 0, ac, hdT2, out_d)

    nc.compile()

    in_maps = []
    bf16 = _bf16()
    eye16 = np.eye(P, dtype=bf16)
    for c in range(NCORES):
        m = dict(
            mdov=plan0.mdov[c],
            ident=eye16,
            ones=np.ones((1, P), dtype=bf16),
            gidx1=_wrap_idx16(plan1, c),
            md1=plan1.md[c],
            sp2d=meta["sp2d"][c],
        )
        for t in range(plan0.n_tiles):
            m[f"bandd{t}"] = meta["bandd"][c][t]
            m[f"bandm{t}"] = meta["bandm"][c][t]
        for l in range(3):
            ws, wn, b = meta["weights"][l]
            m[f"ws{l}"] = np.ascontiguousarray(ws.astype(bf16))
            m[f"wn{l}"] = np.ascontiguousarray(wn.astype(bf16))
            m[f"bias{l}"] = np.ascontiguousarray(b[None, :].astype(bf16))
        in_maps.append(m)

    res = run_bass_kernel_spmd(
        nc, in_maps, core_ids=list(range(NCORES)), trace=trace
    )
    if debug_h1:
        return [res.results[c]["out"] for c in range(NCORES)], res, [
            res.results[c]["h1o"] for c in range(NCORES)
        ]
    return [res.results[c]["out"] for c in range(NCORES)], res


def assemble(meta, outs):
    full = np.zeros((NUM_DST[2], OUTW[2]), np.float32)
    for c in range(NCORES):
        full[meta["blocks"][c]["seeds"]] = outs[c]
    return full


def kernel(**inputs) -> np.ndarray:
    meta = build_host(inputs)
    outs, _ = run_device(meta)
    return assemble(meta, outs)


# revision 72
# speedup vs baseline: 1.4580x; 1.0005x over previous
"""DistSAGE 3-layer GraphSAGE forward on 8 TRN2 NeuronCores (Bass/Tile).

Strategy (graph/data parallel, per the DistSAGE recipe):
  - Partition the 512 seed nodes across 8 cores (64 each, LPT-balanced by
    an additive 2-hop cost estimate); build per-core dependency-driven
    blocks on the host (standard DGL block construction, pure index math).
    No inter-core communication; weights replicated.
  - Row-shard the feature table: each core receives compact per-dst-tile
    band tensors (one DRAM tensor per tile -> each load is one fully
    contiguous HBM read; a [128, W] slice of one wide tensor caps at
    ~260 GB/s).  Band tile = [dst rows TRANSPOSED, bf16] + a degree-
    sorted GRID of message rows in fp8e4: chunk k, row d = x[src]/deg of
    the k-th edge of dst d (pre-scaled on host, zero-padded).  Nodes are
    deg-sorted inside each block segment so tiles are degree-homogeneous;
    grid depth is capped at CAP0=11, excess edges spill to a few overflow
    chunks.  fp8 quantization of the (mean-aggregated) messages costs
    ~5e-3 rel err; the self path stays bf16.
  - Aggregation needs NO masks from DRAM and (for grid chunks) no mask
    generation at all: the matmul rhs is the constant identity, so
    aggT[f,d] += msgs_chunk.T @ I accumulates the pre-scaled mean in
    PSUM.  Overflow chunks use a pure one-hot rhs generated on-chip by
    one fused DVE tensor_scalar (colidx == d[p]) from 4 B/slot metadata.
    Each accumulation group owns a full PSUM bank (start=True resets the
    whole bank's has_written -- interleaved groups in one bank corrupt).
  - Y = aggT.T@W_neigh + hdT.T@W_self; dst rows arrive pre-transposed so
    no identity matmuls.  The Y matmuls of tile t-1 are emitted AFTER
    tile t's aggregation matmuls (1-deep software pipeline) so the PE
    never waits on the DVE PSUM->SBUF copy.
  - h1 is stored fp8 in DRAM.  Layer 1 gathers message rows with the
    dma_gather ucode (slot per edge, sources consumer-grouped, NI=512
    calls carrying [lo, hi) row bounds, emitted in readiness order so
    gathers stream under layer-0 compute).  Layer-1 one-hot masks are
    hoisted before the gathers (DVE does them under layer 0); h_dst
    tiles are kept in SBUF in bf16 and transposed by identity matmuls
    (transpose-DMAs on a HWDGE ring stall the whole ring FIFO on their
    h1-write dependency).
  - Layer 2 is a dense mask-matmul sweep over SBUF-resident h2 tiles.
  - DMA rings: Sync carries the band stream (+ final output), Scalar
    carries h1 writes and all consts (weights/metadata upfront; gather
    indices etc. behind the first band tile); gpsimd runs the gathers.
"""

import heapq

import numpy as np

P = 128
NCORES = 8
NUM_DST = (61952, 5632, 512)
FEAT = 256
OUTW = (256, 256, 19)
SEEDS_PER_CORE = NUM_DST[2] // NCORES  # 64
WINDOW = 32768
NI_GATHER = 512  # dma_gather indices per call (layer 1)


def _bf16():
    import ml_dtypes

    return ml_dtypes.bfloat16


def _fp8():
    import ml_dtypes

    return ml_dtypes.float8_e4m3fn


# ---------------------------------------------------------------------------
# Host-side block construction
# ---------------------------------------------------------------------------


def _balance(ids, deg, n_buckets):
    """LPT bin-packing: reorder ids so consecutive 128-groups have ~equal
    total degree (only full 128-groups are balanced)."""
    if n_buckets <= 1 or len(ids) < n_buckets * P:
        return ids
    order = np.argsort(-deg[ids], kind="stable")
    heap = [(0.0, b, 0) for b in range(n_buckets)]
    heapq.heapify(heap)
    buckets = [[] for _ in range(n_buckets)]
    for i in order:
        load, b, cnt = heapq.heappop(heap)
        buckets[b].append(ids[i])
        cnt += 1
        if cnt < P:
            heapq.heappush(heap, (load + deg[ids[i]], b, cnt))
    return np.concatenate([np.asarray(b, dtype=ids.dtype) for b in buckets])


def _seed_partition(esrc0, edst0, esrc1, edst1, esrc2, edst2, deg0, deg1):
    """LPT-balance seeds across cores by an additive 2-hop cost estimate."""
    h = np.zeros(NUM_DST[1], np.float64)
    np.add.at(h, edst1, deg0[esrc1].astype(np.float64))
    cost = np.zeros(NUM_DST[2], np.float64)
    np.add.at(cost, edst2, h[esrc2] + deg1[esrc2].astype(np.float64))
    order = np.argsort(-cost, kind="stable")
    heap = [(0.0, cc, 0) for cc in range(NCORES)]
    heapq.heapify(heap)
    groups = [[] for _ in range(NCORES)]
    for s in order:
        load, cc, cnt = heapq.heappop(heap)
        groups[cc].append(s)
        cnt += 1
        if cnt < SEEDS_PER_CORE:
            heapq.heappush(heap, (load + cost[s], cc, cnt))
    return [np.array(g, dtype=np.int64) for g in groups]


def _block_for_core(seeds, esrc0, edst0, esrc1, edst1, esrc2, edst2,
                    deg0, deg1, deg2):
    # seeds and l1_extra sorted by deg0 (ascending): layer-0 dst tiles
    # become degree-homogeneous, so the grid band layout (chunk k, row d =
    # k-th edge of dst d) pads minimally.
    seeds = seeds[np.argsort(deg0[seeds], kind="stable")]
    pos2 = np.full(NUM_DST[2], -1, np.int32)
    pos2[seeds] = np.arange(SEEDS_PER_CORE, dtype=np.int32)
    sel2 = pos2[edst2] >= 0
    es2, ed2g = esrc2[sel2], edst2[sel2]
    l1_extra = np.setdiff1d(np.unique(es2), seeds)
    l1_extra = l1_extra[np.argsort(deg0[l1_extra], kind="stable")]
    l1_out = np.concatenate([seeds, l1_extra])
    n1 = len(l1_out)

    pos1 = np.full(NUM_DST[1], -1, np.int32)
    pos1[l1_out] = np.arange(n1, dtype=np.int32)
    sel1 = pos1[edst1] >= 0
    es1, ed1g = esrc1[sel1], edst1[sel1]
    ed1 = pos1[ed1g].astype(np.int64)
    inv1 = (1.0 / np.maximum(deg1[ed1g], 1.0)).astype(np.float32)
    # Consumer-grouped l0_extra ordering: [multi-tile srcs | grp0 | grp1 ...]
    # so each layer-1 dst tile's sources sit in a contiguous band of l0 rows
    # -> its gather calls only depend on an early prefix + its own band of
    # h1, enabling overlap of the layer-1 gather under layer-0 compute.
    n1_tiles = -(-n1 // P)
    mask_x = np.ones(len(es1), bool)
    small = es1 < NUM_DST[1]
    mask_x[small] = pos1[es1[small]] < 0
    pr = np.unique(
        np.stack([es1[mask_x], ed1[mask_x] // P], axis=1), axis=0
    )
    srcs_u, first_idx, cnt = np.unique(
        pr[:, 0], return_index=True, return_counts=True
    )
    multi = srcs_u[cnt > 1]
    multi = multi[np.argsort(deg0[multi], kind="stable")]
    segs = [multi]
    single_mask = cnt == 1
    s_srcs = srcs_u[single_mask]
    s_tile = pr[first_idx[single_mask], 1]
    for tt in range(n1_tiles):
        seg = s_srcs[s_tile == tt]
        seg = seg[np.argsort(deg0[seg], kind="stable")]
        segs.append(seg)

    ed2 = pos2[ed2g].astype(np.int64)
    inv2 = (1.0 / np.maximum(deg2[ed2g], 1.0)).astype(np.float32)
    es2l = pos1[es2].astype(np.int64)

    return dict(
        seeds=seeds, l1_out=l1_out, segs=segs, n1=n1,
        e1g=(es1, ed1, inv1),
        e2=(es2l, ed2, inv2),
    )


def _assemble_l0(blocks, esrc0, edst0, deg0):
    """Build per-core l0_out = [l1_out | segs...] (tight packing), padded
    to a common n0_pad with duplicates of row 0 (they carry no edges).
    Fills blocks[c]['l0_out'], 'n0', 'e0' (local), 'e1' (local srcs)."""
    n1_pad = max(-(-b["n1"] // P) for b in blocks) * P
    n0_pad = -(-max(
        b["n1"] + sum(len(s) for s in b["segs"]) for b in blocks
    ) // P) * P
    for b in blocks:
        cat = np.concatenate([b["l1_out"]] + b["segs"])
        l0 = np.zeros(n0_pad, np.int64)
        l0[: len(cat)] = cat
        l0[len(cat) :] = cat[0]
        pos0 = np.full(NUM_DST[0], -1, np.int32)
        pos0[cat] = np.arange(len(cat), dtype=np.int32)
        b["l0_out"] = l0
        b["n0"] = n0_pad

        sel0 = pos0[edst0] >= 0
        es0, ed0g = esrc0[sel0], edst0[sel0]
        ed0 = pos0[ed0g].astype(np.int64)
        inv0 = (1.0 / np.maximum(deg0[ed0g], 1.0)).astype(np.float32)
        b["e0"] = (es0.astype(np.int64), ed0, inv0)
        es1, ed1, inv1 = b["e1g"]
        b["e1"] = (pos0[es1].astype(np.int64), ed1, inv1)
    return n0_pad, n1_pad


def _slots_by_tile(es, ed, inv, n_tiles):
    """Slot-per-edge: per dst tile, edge slots sorted by src row.
    Returns per-tile (srcs, dloc, inv) arrays (dloc = dst index in tile)."""
    tile = ed // P
    order = np.lexsort((es, tile))
    es, ed, inv, tile = es[order], ed[order], inv[order], tile[order]
    starts = np.searchsorted(tile, np.arange(n_tiles))
    ends = np.searchsorted(tile, np.arange(n_tiles) + 1)
    return [
        (es[s:e], (ed[s:e] - t * P).astype(np.int64), inv[s:e])
        for t, (s, e) in enumerate(zip(starts, ends))
    ]


CAP0 = 11  # grid chunks per tile cap; excess edges go to overflow chunks


def _grid_by_tile(es, ed, inv, n_tiles, cap):
    """Grid layout: per dst tile, edge slot = rank*128 + dloc (rank = edge
    index within its dst), capped at `cap` ranks; excess edges spill to an
    overflow list (slot-per-edge with one-hot masks)."""
    tile = ed // P
    order = np.lexsort((es, ed))
    es, ed, inv = es[order], ed[order], inv[order]
    tile = tile[order]
    starts = np.searchsorted(tile, np.arange(n_tiles))
    ends = np.searchsorted(tile, np.arange(n_tiles) + 1)
    out = []
    for t, (s, e) in enumerate(zip(starts, ends)):
        dloc = (ed[s:e] - t * P).astype(np.int64)
        cnt = np.bincount(dloc, minlength=P)
        first = np.concatenate([[0], np.cumsum(cnt)[:-1]])
        rank = np.arange(e - s) - first[dloc]  # edges sorted by dloc
        ing = rank < cap
        out.append(dict(
            gsrc=es[s:e][ing], gslot=rank[ing] * P + dloc[ing],
            ginv=inv[s:e][ing],
            K=int(min(cnt.max(), cap)) if e > s else 0,
            osrc=es[s:e][~ing], od=dloc[~ing], oinv=inv[s:e][~ing],
        ))
    return out


class GatherPlan:
    """Layer 1: continuous slot stream gathered via dma_gather (one slot
    per edge).  Tile t owns stream slots [slot_off[t], slot_off[t]+m[t]);
    chunks are 128-slot groups; a chunk overlapping two tiles gets one
    metadata column per tile.  Calls are tile-aligned (big calls + a small
    tail call per tile) and carry [lo, hi) row bounds so each call only
    depends on the h-table rows it reads."""

    def __init__(self, n_tiles, slot_counts, ni):
        self.ni = ni
        self.cpc = ni // P  # max chunks per big call
        self.n_tiles = n_tiles
        self.m = slot_counts
        self.slot_off = np.concatenate([[0], np.cumsum(slot_counts)]).astype(np.int64)
        total = int(self.slot_off[-1])
        self.n_chunks = -(-total // P)
        self.n_chunks_pad = self.n_chunks
        # tile-aligned call partition: big calls + small tail call per tile
        TAILC = 2
        self.call_sizes = []
        for t in range(n_tiles):
            s = -(-int(self.slot_off[t]) // P)
            e = -(-int(self.slot_off[t + 1]) // P) if t + 1 < n_tiles else self.n_chunks
            if t + 1 == n_tiles:
                e = self.n_chunks
            nch = e - s
            if nch <= 0:
                continue
            if nch > TAILC + 1:
                head = nch - TAILC
                nbig = -(-head // self.cpc)
                base = head // nbig
                rem = head - base * nbig
                self.call_sizes += [base + (1 if i < rem else 0)
                                    for i in range(nbig)]
                self.call_sizes.append(TAILC)
            else:
                self.call_sizes.append(nch)
        assert sum(self.call_sizes) == self.n_chunks
        self.n_calls = len(self.call_sizes)
        self.call_chunk_off = np.concatenate(
            [[0], np.cumsum(self.call_sizes)]
        ).astype(np.int64)
        self.pairs = []
        self.tile_pairs = []  # per tile: list of (sp_col, chunk)
        for t in range(n_tiles):
            lo, hi = int(self.slot_off[t]), int(self.slot_off[t + 1])
            ch1 = (hi - 1) // P if hi > lo else lo // P
            tp = []
            for ch in range(lo // P, ch1 + 1):
                tp.append((len(self.pairs), ch))
                self.pairs.append((t, ch))
            self.tile_pairs.append(tp)
        self.n_sp_cols = len(self.pairs)
        self.gidx = []  # [NCORES][128, n_chunks_pad] int64 table rows
        self.md = []  # [NCORES][128, n_sp_cols*2] f32 (d, inv) per slot
        self.call_base = None  # [n_calls] row base (lo) per call
        self.call_hi = None  # [n_calls] exclusive row bound per call

    def compute_call_bounds(self, nrows):
        """Per-call [lo, hi) over all cores, 128-aligned."""
        lo = np.zeros(self.n_calls, np.int64)
        hi = np.zeros(self.n_calls, np.int64)
        for k in range(self.n_calls):
            a, b = int(self.call_chunk_off[k]), int(self.call_chunk_off[k + 1])
            mn, mx = nrows, 0
            for g in self.gidx:
                sl = g[:, a:b]
                mn = min(mn, int(sl.min()))
                mx = max(mx, int(sl.max()))
            lo[k] = (mn // P) * P
            hi[k] = min(nrows, ((mx // P) + 1) * P)
        self.call_base = lo
        self.call_hi = hi


def _fill_gather(plan, per_core_tiles, pad_row):
    total_pad = plan.n_chunks_pad * P
    for c in range(NCORES):
        stream = np.zeros(total_pad, np.int64)
        dstream = np.full(total_pad, -1.0, np.float32)
        istream = np.zeros(total_pad, np.float32)
        for t in range(plan.n_tiles):
            lo, hi = int(plan.slot_off[t]), int(plan.slot_off[t + 1])
            srcs, dloc, inv = per_core_tiles[c][t]
            stream[lo : lo + len(srcs)] = srcs
            stream[lo + len(srcs) : hi] = pad_row[c][t]
            dstream[lo : lo + len(srcs)] = dloc
            istream[lo : lo + len(srcs)] = inv
        tail = int(plan.slot_off[-1])
        stream[tail:] = pad_row[c][plan.n_tiles - 1]
        plan.gidx.append(stream.reshape(plan.n_chunks_pad, P).T.copy())

        # metadata: per (tile, chunk) pair one (d, inv) column; slots of
        # the chunk outside the tile's range get (-1, 0).
        md = np.zeros((P, plan.n_sp_cols, 2), np.float32)
        md[:, :, 0] = -1.0
        for t in range(plan.n_tiles):
            lo, hi = int(plan.slot_off[t]), int(plan.slot_off[t + 1])
            for sp_col, ch in plan.tile_pairs[t]:
                s0 = ch * P
                a = max(s0, lo)
                b = min(s0 + P, hi)
                if a < b:
                    md[a - s0 : b - s0, sp_col, 0] = dstream[a:b]
                    md[a - s0 : b - s0, sp_col, 1] = istream[a:b]
        plan.md.append(np.ascontiguousarray(md.reshape(P, plan.n_sp_cols * 2)))


class BandPlan:
    """Layer 0: per-tile dense bands, pre-interleaved.  Tile t's band =
    group 0 (dst rows TRANSPOSED: [f0..127 x d | f128..255 x d]) +
    K[t] grid groups (chunk k row d = x[src]/deg of the k-th edge of dst
    d, pre-scaled on host, zero-padded -> aggregation rhs is the constant
    identity) + O[t] overflow groups (slot-per-edge, pre-scaled, one-hot
    is_equal masks from per-slot dst metadata)."""

    def __init__(self, n_tiles, K, O):
        self.n_tiles = n_tiles
        self.K = [max(1, k) for k in K]  # capped max-over-core degree
        self.O = O  # overflow chunks per tile (max over cores)
        self.goff = np.concatenate(
            [[0], np.cumsum([1 + k + o for k, o in zip(self.K, O)])]
        ).astype(np.int64)
        self.n_groups = int(self.goff[-1])
        self.ov_off = np.concatenate([[0], np.cumsum(O)]).astype(np.int64)
        self.n_ov_cols = int(self.ov_off[-1])
        self.mdov = []  # [NCORES][128, n_ov_cols] f32 dst-id per slot


def build_host(inputs):
    esrc0 = np.asarray(inputs["esrc0"]).astype(np.int64)
    edst0 = np.asarray(inputs["edst0"]).astype(np.int64)
    esrc1 = np.asarray(inputs["esrc1"]).astype(np.int64)
    edst1 = np.asarray(inputs["edst1"]).astype(np.int64)
    esrc2 = np.asarray(inputs["esrc2"]).astype(np.int64)
    edst2 = np.asarray(inputs["edst2"]).astype(np.int64)
    x = np.asarray(inputs["x"], dtype=np.float32)

    deg0 = np.bincount(edst0, minlength=NUM_DST[0]).astype(np.float32)
    deg1 = np.bincount(edst1, minlength=NUM_DST[1]).astype(np.float32)
    deg2 = np.bincount(edst2, minlength=NUM_DST[2]).astype(np.float32)

    seed_groups = _seed_partition(esrc0, edst0, esrc1, edst1, esrc2, edst2,
                                  deg0, deg1)
    blocks = [
        _block_for_core(seed_groups[c], esrc0, edst0, esrc1, edst1, esrc2,
                        edst2, deg0, deg1, deg2)
        for c in range(NCORES)
    ]
    n0_pad, n1_pad = _assemble_l0(blocks, esrc0, edst0, deg0)
    T0, T1, T2 = n0_pad // P, n1_pad // P, 1

    tiles0 = [_grid_by_tile(*b["e0"], T0, CAP0) for b in blocks]
    tiles1 = [_slots_by_tile(*b["e1"], T1) for b in blocks]

    # ---- layer 0: band plan + pre-interleaved compact tables ----
    plan0 = BandPlan(
        T0,
        [max(tiles0[c][t]["K"] for c in range(NCORES)) for t in range(T0)],
        [max(-(-len(tiles0[c][t]["osrc"]) // P) for c in range(NCORES))
         for t in range(T0)],
    )
    l0_padded = [b["l0_out"] for b in blocks]

    bf16 = _bf16()
    fp8 = _fp8()
    x16 = x.astype(bf16)
    # per-core, per-tile: dst groups (bf16, transposed) + msg grids (fp8)
    bandd, bandm = [], []
    for c in range(NCORES):
        bd, bm = [], []
        mdov = np.full((P, max(plan0.n_ov_cols, 1)), -1.0, np.float32)
        for t in range(T0):
            hd = x16[l0_padded[c][t * P : (t + 1) * P]]  # [128 dst, 256 f]
            bd.append(np.ascontiguousarray(
                hd.T.reshape(2, P, P).transpose(1, 0, 2).reshape(P, FEAT)
            ))
            # bd[t][p, h*128+j] = hd[j, h*128+p]
            ti = tiles0[c][t]
            K, O = plan0.K[t], plan0.O[t]
            grid = np.zeros(((K + O) * P, FEAT), np.float32)
            grid[ti["gslot"]] = x[ti["gsrc"]] * ti["ginv"][:, None]
            if O:
                no = len(ti["osrc"])
                grid[K * P : K * P + no] = x[ti["osrc"]] * ti["oinv"][:, None]
                oo = int(plan0.ov_off[t])
                dcol = np.full(O * P, -1.0, np.float32)
                dcol[:no] = ti["od"]
                mdov[:, oo : oo + O] = dcol.reshape(O, P).T
            g8 = grid.astype(fp8).reshape(K + O, P, FEAT)
            bm.append(np.ascontiguousarray(
                g8.transpose(1, 0, 2).reshape(P, (K + O) * FEAT)
            ))
        bandd.append(bd)
        bandm.append(bm)
        plan0.mdov.append(np.ascontiguousarray(mdov))

    # ---- layer 1: gather plan (slot per edge) ----
    m1 = [
        max(1, max(len(tiles1[c][t][0]) for c in range(NCORES)))
        for t in range(T1)
    ]
    plan1 = GatherPlan(T1, m1, NI_GATHER)
    padL = [[t * P for t in range(T1)] for _ in range(NCORES)]
    _fill_gather(plan1, tiles1, padL)
    plan1.compute_call_bounds(n0_pad)
    assert n0_pad <= WINDOW and n1_pad <= WINDOW

    # ---- layer 2: dense sweep over SBUF-resident h2 (no gather) ----
    # sp2dense[c][j] = [128, 128] mask: W[row, seed] = sum inv2 over edges
    # (src local j*128+row -> seed).
    sp2d = []
    for c in range(NCORES):
        es, ed, inv = blocks[c]["e2"]
        W = np.zeros((T1, P, P), np.float32)
        np.add.at(W, (es // P, es % P, ed), inv)
        sp2d.append(
            np.ascontiguousarray(
                W.transpose(1, 0, 2).reshape(P, T1 * P).astype(bf16)
            )
        )

    return dict(
        plan0=plan0,
        plans=(plan1,),
        sp2d=sp2d,
        T=(T0, T1, T2),
        n0_pad=n0_pad,
        n1_pad=n1_pad,
        bandd=bandd,
        bandm=bandm,
        blocks=blocks,
        weights=tuple(
            (
                np.asarray(inputs[f"W_self{l}"], np.float32),
                np.asarray(inputs[f"W_neigh{l}"], np.float32),
                np.asarray(inputs[f"b{l}"], np.float32),
            )
            for l in range(3)
        ),
    )


# ---------------------------------------------------------------------------
# Numpy simulation of the device kernel (validation aid; fp32 stand-in)
# ---------------------------------------------------------------------------


def simulate_core(meta, c):
    plan0 = meta["plan0"]
    colidx = np.arange(P, dtype=np.float32)

    mdov = plan0.mdov[c]
    ws, wn, b = meta["weights"][0]
    table = np.zeros((plan0.n_tiles * P, OUTW[0]), np.float32)
    for t in range(plan0.n_tiles):
        K, O = plan0.K[t], plan0.O[t]
        hdT = meta["bandd"][c][t].astype(np.float32)
        hd = np.concatenate([hdT[:, 0:P].T, hdT[:, P : 2 * P].T], axis=1)
        xm = meta["bandm"][c][t].astype(np.float32).reshape(P, K + O, FEAT)
        mean = xm[:, 0:K, :].sum(axis=1)  # [d, f]
        oo = int(plan0.ov_off[t])
        for o in range(O):
            msgs = xm[:, K + o, :]
            onehot = colidx[None, :] == mdov[:, oo + o : oo + o + 1]
            mean += (msgs.T @ onehot).T
        table[t * P : (t + 1) * P] = np.maximum(hd @ ws + mean @ wn + b, 0.0)

    plan = meta["plans"][0]
    md1 = plan.md[c].reshape(P, plan.n_sp_cols, 2)
    table8 = table.astype(_fp8()).astype(np.float32)  # h1buf is fp8
    ws, wn, b = meta["weights"][1]
    out = np.zeros((plan.n_tiles * P, OUTW[1]), np.float32)
    for t in range(plan.n_tiles):
        hd = table[t * P : (t + 1) * P]
        aggT = np.zeros((FEAT, P), np.float32)
        for sp_col, ch in plan.tile_pairs[t]:
            msgs = table8[plan.gidx[c][:, ch]]
            mask = (colidx[None, :] == md1[:, sp_col, 0:1]) * md1[:, sp_col, 1:2]
            aggT += msgs.T @ mask
        out[t * P : (t + 1) * P] = np.maximum(hd @ ws + aggT.T @ wn + b, 0.0)
    table = out

    # layer 2: dense sweep
    ws, wn, b = meta["weights"][2]
    sp2 = meta["sp2d"][c].astype(np.float32).reshape(P, -1, P)
    hd = table[0:P]
    aggT = np.zeros((FEAT, P), np.float32)
    for j in range(sp2.shape[1]):
        aggT += table[j * P : (j + 1) * P].T @ sp2[:, j, :]
    y = hd @ ws + aggT.T @ wn + b
    return y[:SEEDS_PER_CORE]


# ---------------------------------------------------------------------------
# Device kernel
# ---------------------------------------------------------------------------


def _wrap_idx16(plan, c):
    bases = np.zeros(plan.n_chunks_pad, np.int64)
    for k in range(plan.n_calls):
        bases[plan.call_chunk_off[k] : plan.call_chunk_off[k + 1]] = plan.call_base[k]
    rel = plan.gidx[c] - bases[None, :]
    total16 = plan.n_chunks_pad * P // 16
    out = np.zeros((P, total16), np.int16)
    off16 = 0
    for k in range(plan.n_calls):
        a, b = int(plan.call_chunk_off[k]), int(plan.call_chunk_off[k + 1])
        flat = rel[:, a:b].T.reshape(-1)
        w = flat.reshape(len(flat) // 16, 16).T.astype(np.int16)
        out[:16, off16 : off16 + w.shape[1]] = w
        off16 += w.shape[1]
    for rep in range(1, 8):
        out[rep * 16 : (rep + 1) * 16] = out[:16]
    return out


def run_device(meta, trace=False, debug_h1=False):
    import concourse.bacc as bacc
    import concourse.tile as tile
    import concourse.mybir as mybir
    from concourse.bass_utils import run_bass_kernel_spmd

    plan0 = meta["plan0"]
    plan1 = meta["plans"][0]
    T1 = meta["T"][1]
    f32 = mybir.dt.float32
    i32 = mybir.dt.int32
    b16 = mybir.dt.bfloat16
    f8 = mybir.dt.float8e4
    alu = mybir.AluOpType

    nc = bacc.Bacc("TRN2", target_bir_lowering=False, debug=False, num_devices=NCORES)

    # one DRAM tensor per band tile: the transfer is then one fully
    # contiguous HBM region (a [P, W] slice of a wide tensor reads 128
    # scattered ~7KB segments and caps at ~260 GB/s).  dst groups bf16,
    # message grids fp8 (mean-aggregated -> quantization error washes out)
    bandd_d = [
        nc.dram_tensor(f"bandd{t}", [P, FEAT], b16, kind="ExternalInput")
        for t in range(plan0.n_tiles)
    ]
    bandm_d = [
        nc.dram_tensor(
            f"bandm{t}",
            [P, (plan0.K[t] + plan0.O[t]) * FEAT],
            f8,
            kind="ExternalInput",
        )
        for t in range(plan0.n_tiles)
    ]
    mdov_d = nc.dram_tensor("mdov", [P, max(plan0.n_ov_cols, 1)], f32,
                            kind="ExternalInput")
    ident_d = nc.dram_tensor("ident", [P, P], b16, kind="ExternalInput")
    ones_d = nc.dram_tensor("ones", [1, P], b16, kind="ExternalInput")
    h1buf = nc.dram_tensor("h1buf", [meta["n0_pad"], FEAT], f8)
    out_d = nc.dram_tensor("out", [SEEDS_PER_CORE, OUTW[2]], f32, kind="ExternalOutput")

    h1o_d = None
    if debug_h1:
        h1o_d = nc.dram_tensor("h1o", [meta["n0_pad"], FEAT], f32,
                               kind="ExternalOutput")
    idx1_d = nc.dram_tensor("gidx1", [P, plan1.n_chunks_pad * P // 16],
                            mybir.dt.int16, kind="ExternalInput")
    md1_d = nc.dram_tensor("md1", [P, plan1.n_sp_cols * 2], f32,
                           kind="ExternalInput")
    sp2_d = nc.dram_tensor("sp2d", [P, T1 * P], b16, kind="ExternalInput")
    w_d = []
    for l in range(3):
        w_d.append(
            (
                nc.dram_tensor(f"ws{l}", [FEAT, OUTW[l]], b16, kind="ExternalInput"),
                nc.dram_tensor(f"wn{l}", [FEAT, OUTW[l]], b16, kind="ExternalInput"),
                nc.dram_tensor(f"bias{l}", [1, OUTW[l]], b16, kind="ExternalInput"),
            )
        )

    use_bias = [bool(np.any(meta["weights"][l][2] != 0)) for l in range(3)]

    with tile.TileContext(nc) as tc:
        with (
            tc.tile_pool(name="const", bufs=1) as cpool,
            tc.tile_pool(name="msgs", bufs=9) as mpool,
            tc.tile_pool(name="dsts", bufs=9) as dpool,
            tc.tile_pool(name="mask", bufs=28) as kpool,
            tc.tile_pool(name="acc", bufs=3) as apool,
            tc.tile_pool(name="outp", bufs=3) as opool,
            tc.tile_pool(name="hdt", bufs=3) as hpool,
            tc.tile_pool(name="gmsg", bufs=1) as gpool,
            tc.tile_pool(name="pagg", bufs=2, space="PSUM") as pa,
            tc.tile_pool(name="py", bufs=2, space="PSUM") as pypool,
        ):
            # ---- upfront consts ----
            ident_t = cpool.tile([P, P], b16, tag="ident")
            nc.scalar.dma_start(out=ident_t[:], in_=ident_d[:])
            mdov_t = cpool.tile([P, max(plan0.n_ov_cols, 1)], f32, tag="mdov")
            nc.scalar.dma_start(out=mdov_t[:], in_=mdov_d[:])
            colidx_i = cpool.tile([P, P], i32, tag="colidx_i")
            nc.gpsimd.iota(colidx_i[:], [[1, P]], channel_multiplier=0)
            colidx = cpool.tile([P, P], f32, tag="colidx")
            nc.vector.tensor_copy(out=colidx[:], in_=colidx_i[:])

            ws_ts, wn_ts, bias_ts = [[None, None] for _ in range(3)], \
                [[None, None] for _ in range(3)], [None] * 3
            ones_t = cpool.tile([1, P], b16, tag="ones")

            def load_weights(l, eng):
                outw = OUTW[l]
                for k in range(2):
                    w = cpool.tile([P, outw], b16, tag=f"ws{l}_{k}")
                    eng.dma_start(out=w[:], in_=w_d[l][0][k * P : (k + 1) * P, :])
                    ws_ts[l][k] = w
                    w = cpool.tile([P, outw], b16, tag=f"wn{l}_{k}")
                    eng.dma_start(out=w[:], in_=w_d[l][1][k * P : (k + 1) * P, :])
                    wn_ts[l][k] = w
                if use_bias[l]:
                    bias_t = cpool.tile([1, outw], b16, tag=f"bias{l}")
                    eng.dma_start(out=bias_t[:], in_=w_d[l][2][:])
                    bias_ts[l] = bias_t

            load_weights(0, nc.scalar)
            if any(use_bias):
                nc.scalar.dma_start(out=ones_t[:], in_=ones_d[:])

            h2res = [
                cpool.tile([P, FEAT], b16, tag=f"h2res_{t}", name=f"h2res_{t}")
                for t in range(T1)
            ]
            h1res = [
                cpool.tile([P, FEAT], b16, tag=f"h1res_{t}", name=f"h1res_{t}")
                for t in range(T1)
            ]

            def gen_mask(md_t, col):
                """One-hot mask [128 slots, 128 dst] = (colidx==d[p]) * inv[p]."""
                mk = kpool.tile([P, P], b16, tag="mk")
                nc.vector.tensor_scalar(
                    out=mk[:],
                    in0=colidx[:],
                    scalar1=md_t[:, 2 * col : 2 * col + 1],
                    scalar2=md_t[:, 2 * col + 1 : 2 * col + 2],
                    op0=alu.is_equal,
                    op1=alu.mult,
                )
                return mk

            def gen_mask_ov(col):
                """Pure one-hot [128 slots, 128 dst] = (colidx==d[p])."""
                mk = kpool.tile([P, P], b16, tag="mk")
                nc.vector.tensor_scalar(
                    out=mk[:],
                    in0=colidx[:],
                    scalar1=mdov_t[:, col : col + 1],
                    scalar2=None,
                    op0=alu.is_equal,
                )
                return mk

            def tile_tail(l, t, ac, hdT, dest):
                """Y matmuls + bias + activation + store for one dst tile.
                ac = aggT halves [f-half, d] bf16; hdT = dst rows transposed."""
                outw = OUTW[l]
                y = pypool.tile([P, outw], f32, tag="y")
                nc.tensor.matmul(y[:], lhsT=ac[:, 0:P], rhs=wn_ts[l][0][:],
                                 start=True, stop=False)
                nc.tensor.matmul(y[:], lhsT=ac[:, P : 2 * P], rhs=wn_ts[l][1][:],
                                 start=False, stop=False)
                nc.tensor.matmul(y[:], lhsT=hdT[:, 0:P], rhs=ws_ts[l][0][:],
                                 start=False, stop=False)
                nc.tensor.matmul(y[:], lhsT=hdT[:, P : 2 * P], rhs=ws_ts[l][1][:],
                                 start=False, stop=not use_bias[l])
                if use_bias[l]:
                    nc.tensor.matmul(y[:], lhsT=ones_t[0:1, :],
                                     rhs=bias_ts[l][0:1, :],
                                     start=False, stop=True)
                if l == 0:
                    o2 = opool.tile([P, outw], f8, tag="o2")
                    nc.scalar.activation(
                        out=o2[:], in_=y[:],
                        func=mybir.ActivationFunctionType.Relu,
                    )
                    nc.scalar.dma_start(out=dest[t * P : (t + 1) * P, :], in_=o2[:])
                    if t < T1:
                        # bf16 copy kept on-chip for the layer-1 self path
                        nc.scalar.activation(
                            out=h1res[t][:], in_=y[:],
                            func=mybir.ActivationFunctionType.Relu,
                        )
                    if debug_h1:
                        od = opool.tile([P, outw], f32, tag="od")
                        nc.vector.tensor_copy(out=od[:], in_=o2[:])
                        nc.sync.dma_start(
                            out=h1o_d[t * P : (t + 1) * P, :], in_=od[:]
                        )
                elif l == 1:
                    nc.scalar.activation(
                        out=h2res[t][:], in_=y[:],
                        func=mybir.ActivationFunctionType.Relu,
                    )
                else:
                    o = opool.tile([P, outw], f32, tag="o")
                    nc.vector.tensor_copy(out=o[:], in_=y[:])
                    nc.sync.dma_start(out=dest[:], in_=o[0:SEEDS_PER_CORE, :])

            # ================= layer 0: dense bands =================
            # 1-deep software pipeline: tile t's mask matmuls are emitted
            # before tile t-1's PSUM copy + Y matmuls, so the PE never
            # waits on the DVE copy.
            Kmax = max(k + o for k, o in zip(plan0.K, plan0.O))
            pending = None  # (t, pc0, pc1, hdT_view)
            for t in range(plan0.n_tiles):
                K, O = plan0.K[t], plan0.O[t]
                oo = int(plan0.ov_off[t])
                btd = dpool.tile([P, FEAT], b16, tag="bandd")
                nc.sync.dma_start(out=btd[:], in_=bandd_d[t][:])
                btm = mpool.tile([P, Kmax * FEAT], f8, tag="bandm")
                nc.sync.dma_start(
                    out=btm[:, : (K + O) * FEAT], in_=bandm_d[t][:]
                )
                omasks = [gen_mask_ov(oo + o) for o in range(O)]
                # two PSUM tiles: each accumulation group must own its bank
                # (start=True resets the whole bank's has_written); rhs is
                # the constant identity for grid chunks (rows pre-scaled,
                # dst-aligned), a one-hot mask for overflow chunks
                pc0 = pa.tile([P, P], f32, tag="pc0")
                pc1 = pa.tile([P, P], f32, tag="pc1")
                for k in range(K + O):
                    st, sp = (k == 0), (k == K + O - 1)
                    base = k * FEAT
                    rhs = ident_t[:] if k < K else omasks[k - K][:]
                    nc.tensor.matmul(pc0[:], lhsT=btm[:, base : base + P],
                                     rhs=rhs, start=st, stop=sp)
                    nc.tensor.matmul(pc1[:],
                                     lhsT=btm[:, base + P : base + 2 * P],
                                     rhs=rhs, start=st, stop=sp)
                if pending is not None:
                    tp, pc0p, pc1p, hdTp = pending
                    ac = apool.tile([P, FEAT], b16, tag="ac")
                    nc.vector.tensor_copy(out=ac[:, 0:P], in_=pc0p[:])
                    nc.vector.tensor_copy(out=ac[:, P : 2 * P], in_=pc1p[:])
                    tile_tail(0, tp, ac, hdTp, h1buf)
                pending = (t, pc0, pc1, btd[:])

                if t == 0:
                    # late consts: emitted behind the first band loads so the
                    # main stream starts immediately; all are ready long
                    # before their consumers run.
                    idx1_t = cpool.tile(list(idx1_d.shape), mybir.dt.int16,
                                        tag="idx1")
                    nc.scalar.dma_start(out=idx1_t[:], in_=idx1_d[:])
                    load_weights(1, nc.scalar)
                    load_weights(2, nc.scalar)
                    sp2_t = cpool.tile([P, T1 * P], b16, tag="sp2d")
                    nc.scalar.dma_start(out=sp2_t[:], in_=sp2_d[:])
                    md1_t = cpool.tile([P, plan1.n_sp_cols * 2], f32, tag="md1")
                    nc.scalar.dma_start(out=md1_t[:], in_=md1_d[:])
            # flush the pipeline
            tp, pc0p, pc1p, hdTp = pending
            ac = apool.tile([P, FEAT], b16, tag="ac")
            nc.vector.tensor_copy(out=ac[:, 0:P], in_=pc0p[:])
            nc.vector.tensor_copy(out=ac[:, P : 2 * P], in_=pc1p[:])
            tile_tail(0, tp, ac, hdTp, h1buf)

            # ================= layer 1: overlapped gather =================
            # hoist all layer-1 mask generation (depends only on md1/colidx)
            # so the DVE does it under layer-0 compute instead of in the
            # gather-bound tail (NOT on gpsimd: Q7 tensor_scalar is ~20x
            # slower and serializes ahead of the gather descriptor gen)
            l1_masks = []
            for t in range(plan1.n_tiles):
                tm = []
                for i, (sp_col, _) in enumerate(plan1.tile_pairs[t]):
                    mk = cpool.tile([P, P], b16, tag=f"mk1_{t}_{i}",
                                    name=f"mk1_{t}_{i}")
                    nc.vector.tensor_scalar(
                        out=mk[:],
                        in0=colidx[:],
                        scalar1=md1_t[:, 2 * sp_col : 2 * sp_col + 1],
                        scalar2=md1_t[:, 2 * sp_col + 1 : 2 * sp_col + 2],
                        op0=alu.is_equal,
                        op1=alu.mult,
                    )
                    tm.append(mk)
                l1_masks.append(tm)

            # transposed h_dst tiles via identity matmuls from the
            # SBUF-resident h1res copies (no DMA-ring blocking)
            hdts = []
            for t in range(T1):
                ph = pa.tile([P, FEAT], f32, tag="pht", name=f"pht_{t}")
                nc.tensor.matmul(ph[:, 0:P], lhsT=h1res[t][:, 0:P],
                                 rhs=ident_t[:], start=True, stop=True)
                nc.tensor.matmul(ph[:, P : 2 * P], lhsT=h1res[t][:, P : 2 * P],
                                 rhs=ident_t[:], start=True, stop=True)
                ht = hpool.tile([P, FEAT], b16, tag=f"hdt_{t}", name=f"hdt_{t}")
                nc.vector.tensor_copy(out=ht[:], in_=ph[:])
                hdts.append(ht)

            call_tiles = [None] * plan1.n_calls
            order = sorted(
                range(plan1.n_calls),
                key=lambda k: (int(plan1.call_hi[k]), int(plan1.call_base[k])),
            )
            for k in order:
                a = int(plan1.call_chunk_off[k])
                b2 = int(plan1.call_chunk_off[k + 1])
                sz = b2 - a
                lo = int(plan1.call_base[k])
                hi = int(plan1.call_hi[k])
                mt = gpool.tile([P, sz * FEAT], f8, tag=f"msgs1_{k}")
                nc.gpsimd.dma_gather(
                    out_ap=mt[:, : sz * FEAT].rearrange("p (g d) -> p g d", g=sz),
                    in_ap=h1buf[lo:hi, :],
                    idxs_ap=idx1_t[:, a * P // 16 : b2 * P // 16],
                    num_idxs=sz * P,
                    num_idxs_reg=sz * P,
                    elem_size=FEAT,
                    single_packet=False,
                )
                call_tiles[k] = (mt, a)

            call_of_chunk = np.searchsorted(
                plan1.call_chunk_off, np.arange(plan1.n_chunks_pad), side="right"
            ) - 1

            def msg_slice(ch, f0, f1):
                k = int(call_of_chunk[ch])
                mt, a = call_tiles[k]
                j = ch - a
                return mt[:, j * FEAT + f0 : j * FEAT + f1]

            pending = None
            for t in range(plan1.n_tiles):
                pairs = plan1.tile_pairs[t]
                masks = l1_masks[t]
                pc0 = pa.tile([P, P], f32, tag="pc0")
                pc1 = pa.tile([P, P], f32, tag="pc1")
                for i, (sp_col, ch) in enumerate(pairs):
                    st, sp = (i == 0), (i == len(pairs) - 1)
                    nc.tensor.matmul(pc0[:], lhsT=msg_slice(ch, 0, P),
                                     rhs=masks[i][:], start=st, stop=sp)
                    nc.tensor.matmul(pc1[:],
                                     lhsT=msg_slice(ch, P, 2 * P),
                                     rhs=masks[i][:], start=st, stop=sp)
                if pending is not None:
                    tp, pc0p, pc1p = pending
                    ac = apool.tile([P, FEAT], b16, tag="ac")
                    nc.vector.tensor_copy(out=ac[:, 0:P], in_=pc0p[:])
                    nc.vector.tensor_copy(out=ac[:, P : 2 * P], in_=pc1p[:])
                    tile_tail(1, tp, ac, hdts[tp], None)
                pending = (t, pc0, pc1)
            tp, pc0p, pc1p = pending
            ac = apool.tile([P, FEAT], b16, tag="ac")
            nc.vector.tensor_copy(out=ac[:, 0:P], in_=pc0p[:])
            nc.vector.tensor_copy(out=ac[:, P : 2 * P], in_=pc1p[:])
            tile_tail(1, tp, ac, hdts[tp], None)

            # ================= layer 2: dense sweep over h2res =================
            pc0 = pa.tile([P, P], f32, tag="pc0")
            pc1 = pa.tile([P, P], f32, tag="pc1")
            for j in range(T1):
                st, sp = (j == 0), (j == T1 - 1)
                nc.tensor.matmul(pc0[:], lhsT=h2res[j][:, 0:P],
                                 rhs=sp2_t[:, j * P : (j + 1) * P],
                                 start=st, stop=sp)
                nc.tensor.matmul(pc1[:], lhsT=h2res[j][:, P : 2 * P],
                                 rhs=sp2_t[:, j * P : (j + 1) * P],
                                 start=st, stop=sp)
            # transpose h2res[0] for the self path (identity matmuls)
            ph0 = pa.tile([P, P], f32, tag="pc0")
            ph1 = pa.tile([P, P], f32, tag="pc1")
            nc.tensor.matmul(ph0[:], lhsT=h2res[0][:, 0:P],
                             rhs=ident_t[:], start=True, stop=True)
            nc.tensor.matmul(ph1[:], lhsT=h2res[0][:, P : 2 * P],
                             rhs=ident_t[:], start=True, stop=True)
            hdT2 = apool.tile([P, FEAT], b16, tag="hdT2")
            nc.vector.tensor_copy(out=hdT2[:, 0:P], in_=ph0[:])
            nc.vector.tensor_copy(out=hdT2[:, P : 2 * P], in_=ph1[:])
            ac = apool.tile([P, FEAT], b16, tag="ac")
            nc.vector.tensor_copy(out=ac[:, 0:P], in_=pc0[:])
            nc.vector.tensor_copy(out=ac[:, P : 2 * P], in_=pc1[:])
            tile_tail(2, 0, ac, hdT2, out_d)

    nc.compile()

    in_maps = []
    bf16 = _bf16()
    eye16 = np.eye(P, dtype=bf16)
    for c in range(NCORES):
        m = dict(
            mdov=plan0.mdov[c],
            ident=eye16,
            ones=np.ones((1, P), dtype=bf16),
            gidx1=_wrap_idx16(plan1, c),
            md1=plan1.md[c],
            sp2d=meta["sp2d"][c],
        )
        for t in range(plan0.n_tiles):
            m[f"bandd{t}"] = meta["bandd"][c][t]
            m[f"bandm{t}"] = meta["bandm"][c][t]
        for l in range(3):
            ws, wn, b = meta["weights"][l]
            m[f"ws{l}"] = np.ascontiguousarray(ws.astype(bf16))
            m[f"wn{l}"] = np.ascontiguousarray(wn.astype(bf16))
            m[f"bias{l}"] = np.ascontiguousarray(b[None, :].astype(bf16))
        in_maps.append(m)

    res = run_bass_kernel_spmd(
        nc, in_maps, core_ids=list(range(NCORES)), trace=trace
    )
    if debug_h1:
        return [res.results[c]["out"] for c in range(NCORES)], res, [
            res.results[c]["h1o"] for c in range(NCORES)
        ]
    return [res.results[c]["out"] for c in range(NCORES)], res


def assemble(meta, outs):
    full = np.zeros((NUM_DST[2], OUTW[2]), np.float32)
    for c in range(NCORES):
        full[meta["blocks"][c]["seeds"]] = outs[c]
    return full


def kernel(**inputs) -> np.ndarray:
    meta = build_host(inputs)
    outs, _ = run_device(meta)
    return assemble(meta, outs)


# revision 77
# speedup vs baseline: 1.5448x; 1.0596x over previous
"""DistSAGE 3-layer GraphSAGE forward on 8 TRN2 NeuronCores (Bass/Tile).

Strategy (graph/data parallel, per the DistSAGE recipe):
  - Partition the 512 seed nodes across 8 cores (64 each, LPT-balanced by
    an additive 2-hop cost estimate); build per-core dependency-driven
    blocks on the host (standard DGL block construction, pure index math).
    No inter-core communication; weights replicated.
  - Row-shard the feature table: each core receives compact per-dst-tile
    band tensors (one DRAM tensor per tile -> each load is one fully
    contiguous HBM read; a [128, W] slice of one wide tensor caps at
    ~260 GB/s).  Band tile = [dst rows TRANSPOSED, bf16] + a degree-
    sorted GRID of message rows in fp8e4: chunk k, row d = x[src]/deg of
    the k-th edge of dst d (pre-scaled on host, zero-padded).  Nodes are
    deg-sorted inside each block segment so tiles are degree-homogeneous;
    grid depth is capped at CAP0=11, excess edges spill to a few overflow
    chunks.  fp8 quantization of the (mean-aggregated) messages costs
    ~5e-3 rel err; the self path stays bf16.
  - Aggregation needs NO masks from DRAM and (for grid chunks) no mask
    generation at all: the matmul rhs is the constant identity, so
    aggT[f,d] += msgs_chunk.T @ I accumulates the pre-scaled mean in
    PSUM.  Overflow chunks use a pure one-hot rhs generated on-chip by
    one fused DVE tensor_scalar (colidx == d[p]) from 4 B/slot metadata.
    Each accumulation group owns a full PSUM bank (start=True resets the
    whole bank's has_written -- interleaved groups in one bank corrupt).
  - Y = aggT.T@W_neigh + hdT.T@W_self; dst rows arrive pre-transposed so
    no identity matmuls.  The Y matmuls of tile t-1 are emitted AFTER
    tile t's aggregation matmuls (1-deep software pipeline) so the PE
    never waits on the DVE PSUM->SBUF copy.
  - h1 is stored fp8 in DRAM.  Layer 1 gathers message rows with the
    dma_gather ucode (slot per edge, sources consumer-grouped, NI=512
    calls carrying [lo, hi) row bounds, emitted in readiness order so
    gathers stream under layer-0 compute).  Layer-1 one-hot masks are
    hoisted before the gathers (DVE does them under layer 0); h_dst
    tiles are kept in SBUF in bf16 and transposed by identity matmuls
    (transpose-DMAs on a HWDGE ring stall the whole ring FIFO on their
    h1-write dependency).
  - Layer 2 is a dense mask-matmul sweep over SBUF-resident h2 tiles.
  - DMA rings: Sync carries the band stream (+ final output), Scalar
    carries h1 writes and all consts (weights/metadata upfront; gather
    indices etc. behind the first band tile); gpsimd runs the gathers.
"""

import heapq

import numpy as np

P = 128
NCORES = 8
NUM_DST = (61952, 5632, 512)
FEAT = 256
OUTW = (256, 256, 19)
SEEDS_PER_CORE = NUM_DST[2] // NCORES  # 64
WINDOW = 32768
NI_GATHER = 512  # dma_gather indices per call (layer 1)


def _bf16():
    import ml_dtypes

    return ml_dtypes.bfloat16


def _fp8():
    import ml_dtypes

    return ml_dtypes.float8_e4m3fn


# ---------------------------------------------------------------------------
# Host-side block construction
# ---------------------------------------------------------------------------


def _balance(ids, deg, n_buckets):
    """LPT bin-packing: reorder ids so consecutive 128-groups have ~equal
    total degree (only full 128-groups are balanced)."""
    if n_buckets <= 1 or len(ids) < n_buckets * P:
        return ids
    order = np.argsort(-deg[ids], kind="stable")
    heap = [(0.0, b, 0) for b in range(n_buckets)]
    heapq.heapify(heap)
    buckets = [[] for _ in range(n_buckets)]
    for i in order:
        load, b, cnt = heapq.heappop(heap)
        buckets[b].append(ids[i])
        cnt += 1
        if cnt < P:
            heapq.heappush(heap, (load + deg[ids[i]], b, cnt))
    return np.concatenate([np.asarray(b, dtype=ids.dtype) for b in buckets])


def _seed_partition(esrc0, edst0, esrc1, edst1, esrc2, edst2, deg0, deg1):
    """LPT-balance seeds across cores by an additive 2-hop cost estimate."""
    h = np.zeros(NUM_DST[1], np.float64)
    np.add.at(h, edst1, deg0[esrc1].astype(np.float64))
    cost = np.zeros(NUM_DST[2], np.float64)
    np.add.at(cost, edst2, h[esrc2] + deg1[esrc2].astype(np.float64))
    order = np.argsort(-cost, kind="stable")
    heap = [(0.0, cc, 0) for cc in range(NCORES)]
    heapq.heapify(heap)
    groups = [[] for _ in range(NCORES)]
    for s in order:
        load, cc, cnt = heapq.heappop(heap)
        groups[cc].append(s)
        cnt += 1
        if cnt < SEEDS_PER_CORE:
            heapq.heappush(heap, (load + cost[s], cc, cnt))
    return [np.array(g, dtype=np.int64) for g in groups]


def _block_for_core(seeds, esrc0, edst0, esrc1, edst1, esrc2, edst2,
                    deg0, deg1, deg2):
    # seeds and l1_extra sorted by deg0 (ascending): layer-0 dst tiles
    # become degree-homogeneous, so the grid band layout (chunk k, row d =
    # k-th edge of dst d) pads minimally.
    seeds = seeds[np.argsort(deg0[seeds], kind="stable")]
    pos2 = np.full(NUM_DST[2], -1, np.int32)
    pos2[seeds] = np.arange(SEEDS_PER_CORE, dtype=np.int32)
    sel2 = pos2[edst2] >= 0
    es2, ed2g = esrc2[sel2], edst2[sel2]
    l1_extra = np.setdiff1d(np.unique(es2), seeds)
    l1_extra = l1_extra[np.argsort(deg0[l1_extra], kind="stable")]
    l1_out = np.concatenate([seeds, l1_extra])
    n1 = len(l1_out)

    pos1 = np.full(NUM_DST[1], -1, np.int32)
    pos1[l1_out] = np.arange(n1, dtype=np.int32)
    sel1 = pos1[edst1] >= 0
    es1, ed1g = esrc1[sel1], edst1[sel1]
    ed1 = pos1[ed1g].astype(np.int64)
    inv1 = (1.0 / np.maximum(deg1[ed1g], 1.0)).astype(np.float32)
    # Consumer-grouped l0_extra ordering: [multi-tile srcs | grp0 | grp1 ...]
    # so each layer-1 dst tile's sources sit in a contiguous band of l0 rows
    # -> its gather calls only depend on an early prefix + its own band of
    # h1, enabling overlap of the layer-1 gather under layer-0 compute.
    n1_tiles = -(-n1 // P)
    mask_x = np.ones(len(es1), bool)
    small = es1 < NUM_DST[1]
    mask_x[small] = pos1[es1[small]] < 0
    pr = np.unique(
        np.stack([es1[mask_x], ed1[mask_x] // P], axis=1), axis=0
    )
    srcs_u, first_idx, cnt = np.unique(
        pr[:, 0], return_index=True, return_counts=True
    )
    multi = srcs_u[cnt > 1]
    multi = multi[np.argsort(deg0[multi], kind="stable")]
    segs = [multi]
    single_mask = cnt == 1
    s_srcs = srcs_u[single_mask]
    s_tile = pr[first_idx[single_mask], 1]
    for tt in range(n1_tiles):
        seg = s_srcs[s_tile == tt]
        seg = seg[np.argsort(deg0[seg], kind="stable")]
        segs.append(seg)

    ed2 = pos2[ed2g].astype(np.int64)
    inv2 = (1.0 / np.maximum(deg2[ed2g], 1.0)).astype(np.float32)
    es2l = pos1[es2].astype(np.int64)

    return dict(
        seeds=seeds, l1_out=l1_out, segs=segs, n1=n1,
        e1g=(es1, ed1, inv1),
        e2=(es2l, ed2, inv2),
    )


def _assemble_l0(blocks, esrc0, edst0, deg0):
    """Build per-core l0_out = [l1_out | segs...] (tight packing), padded
    to a common n0_pad with duplicates of row 0 (they carry no edges).
    Fills blocks[c]['l0_out'], 'n0', 'e0' (local), 'e1' (local srcs)."""
    n1_pad = max(-(-b["n1"] // P) for b in blocks) * P
    n0_pad = -(-max(
        b["n1"] + sum(len(s) for s in b["segs"]) for b in blocks
    ) // P) * P
    for b in blocks:
        cat = np.concatenate([b["l1_out"]] + b["segs"])
        l0 = np.zeros(n0_pad, np.int64)
        l0[: len(cat)] = cat
        l0[len(cat) :] = cat[0]
        pos0 = np.full(NUM_DST[0], -1, np.int32)
        pos0[cat] = np.arange(len(cat), dtype=np.int32)
        b["l0_out"] = l0
        b["n0"] = n0_pad

        sel0 = pos0[edst0] >= 0
        es0, ed0g = esrc0[sel0], edst0[sel0]
        ed0 = pos0[ed0g].astype(np.int64)
        inv0 = (1.0 / np.maximum(deg0[ed0g], 1.0)).astype(np.float32)
        b["e0"] = (es0.astype(np.int64), ed0, inv0)
        es1, ed1, inv1 = b["e1g"]
        b["e1"] = (pos0[es1].astype(np.int64), ed1, inv1)
    return n0_pad, n1_pad


def _slots_by_tile(es, ed, inv, n_tiles):
    """Slot-per-edge: per dst tile, edge slots sorted by src row.
    Returns per-tile (srcs, dloc, inv) arrays (dloc = dst index in tile)."""
    tile = ed // P
    order = np.lexsort((es, tile))
    es, ed, inv, tile = es[order], ed[order], inv[order], tile[order]
    starts = np.searchsorted(tile, np.arange(n_tiles))
    ends = np.searchsorted(tile, np.arange(n_tiles) + 1)
    return [
        (es[s:e], (ed[s:e] - t * P).astype(np.int64), inv[s:e])
        for t, (s, e) in enumerate(zip(starts, ends))
    ]


CAP0 = 11  # grid chunks per tile cap; excess edges go to overflow chunks


def _grid_by_tile(es, ed, inv, n_tiles, cap):
    """Grid layout: per dst tile, edge slot = rank*128 + dloc (rank = edge
    index within its dst), capped at `cap` ranks; excess edges spill to an
    overflow list (slot-per-edge with one-hot masks)."""
    tile = ed // P
    order = np.lexsort((es, ed))
    es, ed, inv = es[order], ed[order], inv[order]
    tile = tile[order]
    starts = np.searchsorted(tile, np.arange(n_tiles))
    ends = np.searchsorted(tile, np.arange(n_tiles) + 1)
    out = []
    for t, (s, e) in enumerate(zip(starts, ends)):
        dloc = (ed[s:e] - t * P).astype(np.int64)
        cnt = np.bincount(dloc, minlength=P)
        first = np.concatenate([[0], np.cumsum(cnt)[:-1]])
        rank = np.arange(e - s) - first[dloc]  # edges sorted by dloc
        ing = rank < cap
        out.append(dict(
            gsrc=es[s:e][ing], gslot=rank[ing] * P + dloc[ing],
            ginv=inv[s:e][ing],
            K=int(min(cnt.max(), cap)) if e > s else 0,
            osrc=es[s:e][~ing], od=dloc[~ing], oinv=inv[s:e][~ing],
        ))
    return out


class GatherPlan:
    """Layer 1: continuous slot stream gathered via dma_gather (one slot
    per edge).  Tile t owns stream slots [slot_off[t], slot_off[t]+m[t]);
    chunks are 128-slot groups; a chunk overlapping two tiles gets one
    metadata column per tile.  Calls are tile-aligned (big calls + a small
    tail call per tile) and carry [lo, hi) row bounds so each call only
    depends on the h-table rows it reads."""

    def __init__(self, n_tiles, slot_counts, ni):
        self.ni = ni
        self.cpc = ni // P  # max chunks per big call
        self.n_tiles = n_tiles
        self.m = slot_counts
        self.slot_off = np.concatenate([[0], np.cumsum(slot_counts)]).astype(np.int64)
        total = int(self.slot_off[-1])
        self.n_chunks = -(-total // P)
        self.n_chunks_pad = self.n_chunks
        # tile-aligned call partition: big calls + small tail call per tile
        TAILC = 2
        self.call_sizes = []
        for t in range(n_tiles):
            s = -(-int(self.slot_off[t]) // P)
            e = -(-int(self.slot_off[t + 1]) // P) if t + 1 < n_tiles else self.n_chunks
            if t + 1 == n_tiles:
                e = self.n_chunks
            nch = e - s
            if nch <= 0:
                continue
            if nch > TAILC + 1:
                head = nch - TAILC
                nbig = -(-head // self.cpc)
                base = head // nbig
                rem = head - base * nbig
                self.call_sizes += [base + (1 if i < rem else 0)
                                    for i in range(nbig)]
                self.call_sizes.append(TAILC)
            else:
                self.call_sizes.append(nch)
        assert sum(self.call_sizes) == self.n_chunks
        self.n_calls = len(self.call_sizes)
        self.call_chunk_off = np.concatenate(
            [[0], np.cumsum(self.call_sizes)]
        ).astype(np.int64)
        self.pairs = []
        self.tile_pairs = []  # per tile: list of (sp_col, chunk)
        for t in range(n_tiles):
            lo, hi = int(self.slot_off[t]), int(self.slot_off[t + 1])
            ch1 = (hi - 1) // P if hi > lo else lo // P
            tp = []
            for ch in range(lo // P, ch1 + 1):
                tp.append((len(self.pairs), ch))
                self.pairs.append((t, ch))
            self.tile_pairs.append(tp)
        self.n_sp_cols = len(self.pairs)
        self.gidx = []  # [NCORES][128, n_chunks_pad] int64 table rows
        self.md = []  # [NCORES][128, n_sp_cols*2] f32 (d, inv) per slot
        self.call_base = None  # [n_calls] row base (lo) per call
        self.call_hi = None  # [n_calls] exclusive row bound per call

    def compute_call_bounds(self, nrows):
        """Per-call [lo, hi) over all cores, 128-aligned."""
        lo = np.zeros(self.n_calls, np.int64)
        hi = np.zeros(self.n_calls, np.int64)
        for k in range(self.n_calls):
            a, b = int(self.call_chunk_off[k]), int(self.call_chunk_off[k + 1])
            mn, mx = nrows, 0
            for g in self.gidx:
                sl = g[:, a:b]
                mn = min(mn, int(sl.min()))
                mx = max(mx, int(sl.max()))
            lo[k] = (mn // P) * P
            hi[k] = min(nrows, ((mx // P) + 1) * P)
        self.call_base = lo
        self.call_hi = hi


def _fill_gather(plan, per_core_tiles, pad_row):
    total_pad = plan.n_chunks_pad * P
    for c in range(NCORES):
        stream = np.zeros(total_pad, np.int64)
        dstream = np.full(total_pad, -1.0, np.float32)
        istream = np.zeros(total_pad, np.float32)
        for t in range(plan.n_tiles):
            lo, hi = int(plan.slot_off[t]), int(plan.slot_off[t + 1])
            srcs, dloc, inv = per_core_tiles[c][t]
            stream[lo : lo + len(srcs)] = srcs
            stream[lo + len(srcs) : hi] = pad_row[c][t]
            dstream[lo : lo + len(srcs)] = dloc
            istream[lo : lo + len(srcs)] = inv
        tail = int(plan.slot_off[-1])
        stream[tail:] = pad_row[c][plan.n_tiles - 1]
        plan.gidx.append(stream.reshape(plan.n_chunks_pad, P).T.copy())

        # metadata: per (tile, chunk) pair one (d, inv) column; slots of
        # the chunk outside the tile's range get (-1, 0).
        md = np.zeros((P, plan.n_sp_cols, 2), np.float32)
        md[:, :, 0] = -1.0
        for t in range(plan.n_tiles):
            lo, hi = int(plan.slot_off[t]), int(plan.slot_off[t + 1])
            for sp_col, ch in plan.tile_pairs[t]:
                s0 = ch * P
                a = max(s0, lo)
                b = min(s0 + P, hi)
                if a < b:
                    md[a - s0 : b - s0, sp_col, 0] = dstream[a:b]
                    md[a - s0 : b - s0, sp_col, 1] = istream[a:b]
        plan.md.append(np.ascontiguousarray(md.reshape(P, plan.n_sp_cols * 2)))


class BandPlan:
    """Layer 0: per-tile dense bands, pre-interleaved.  Tile t's band =
    group 0 (dst rows TRANSPOSED: [f0..127 x d | f128..255 x d]) +
    K[t] grid groups (chunk k row d = x[src]/deg of the k-th edge of dst
    d, pre-scaled on host, zero-padded -> aggregation rhs is the constant
    identity) + O[t] overflow groups (slot-per-edge, pre-scaled, one-hot
    is_equal masks from per-slot dst metadata)."""

    def __init__(self, n_tiles, K, O):
        self.n_tiles = n_tiles
        self.K = [max(1, k) for k in K]  # capped max-over-core degree
        self.O = O  # overflow chunks per tile (max over cores)
        self.goff = np.concatenate(
            [[0], np.cumsum([1 + k + o for k, o in zip(self.K, O)])]
        ).astype(np.int64)
        self.n_groups = int(self.goff[-1])
        self.ov_off = np.concatenate([[0], np.cumsum(O)]).astype(np.int64)
        self.n_ov_cols = int(self.ov_off[-1])
        self.mdov = []  # [NCORES][128, n_ov_cols] f32 dst-id per slot


def build_host(inputs):
    esrc0 = np.asarray(inputs["esrc0"]).astype(np.int64)
    edst0 = np.asarray(inputs["edst0"]).astype(np.int64)
    esrc1 = np.asarray(inputs["esrc1"]).astype(np.int64)
    edst1 = np.asarray(inputs["edst1"]).astype(np.int64)
    esrc2 = np.asarray(inputs["esrc2"]).astype(np.int64)
    edst2 = np.asarray(inputs["edst2"]).astype(np.int64)
    x = np.asarray(inputs["x"], dtype=np.float32)

    deg0 = np.bincount(edst0, minlength=NUM_DST[0]).astype(np.float32)
    deg1 = np.bincount(edst1, minlength=NUM_DST[1]).astype(np.float32)
    deg2 = np.bincount(edst2, minlength=NUM_DST[2]).astype(np.float32)

    seed_groups = _seed_partition(esrc0, edst0, esrc1, edst1, esrc2, edst2,
                                  deg0, deg1)
    blocks = [
        _block_for_core(seed_groups[c], esrc0, edst0, esrc1, edst1, esrc2,
                        edst2, deg0, deg1, deg2)
        for c in range(NCORES)
    ]
    n0_pad, n1_pad = _assemble_l0(blocks, esrc0, edst0, deg0)
    T0, T1, T2 = n0_pad // P, n1_pad // P, 1

    tiles0 = [_grid_by_tile(*b["e0"], T0, CAP0) for b in blocks]
    tiles1 = [_slots_by_tile(*b["e1"], T1) for b in blocks]

    # ---- layer 0: band plan + pre-interleaved compact tables ----
    plan0 = BandPlan(
        T0,
        [max(tiles0[c][t]["K"] for c in range(NCORES)) for t in range(T0)],
        [max(-(-len(tiles0[c][t]["osrc"]) // P) for c in range(NCORES))
         for t in range(T0)],
    )
    l0_padded = [b["l0_out"] for b in blocks]

    bf16 = _bf16()
    fp8 = _fp8()
    x16 = x.astype(bf16)
    # per-core, per-tile: dst groups (bf16, transposed) + msg grids (fp8)
    bandd, bandm = [], []
    for c in range(NCORES):
        bd, bm = [], []
        mdov = np.full((P, max(plan0.n_ov_cols, 1)), -1.0, np.float32)
        for t in range(T0):
            hd = x16[l0_padded[c][t * P : (t + 1) * P]]  # [128 dst, 256 f]
            bd.append(np.ascontiguousarray(
                hd.T.reshape(2, P, P).transpose(1, 0, 2).reshape(P, FEAT)
            ))
            # bd[t][p, h*128+j] = hd[j, h*128+p]
            ti = tiles0[c][t]
            K, O = plan0.K[t], plan0.O[t]
            grid = np.zeros(((K + O) * P, FEAT), np.float32)
            grid[ti["gslot"]] = x[ti["gsrc"]] * ti["ginv"][:, None]
            if O:
                no = len(ti["osrc"])
                grid[K * P : K * P + no] = x[ti["osrc"]] * ti["oinv"][:, None]
                oo = int(plan0.ov_off[t])
                dcol = np.full(O * P, -1.0, np.float32)
                dcol[:no] = ti["od"]
                mdov[:, oo : oo + O] = dcol.reshape(O, P).T
            g8 = grid.astype(fp8).reshape(K + O, P, FEAT)
            bm.append(np.ascontiguousarray(
                g8.transpose(1, 0, 2).reshape(P, (K + O) * FEAT)
            ))
        bandd.append(bd)
        bandm.append(bm)
        plan0.mdov.append(np.ascontiguousarray(mdov))

    # ---- layer 1: gather plan (slot per edge) ----
    m1 = [
        max(1, max(len(tiles1[c][t][0]) for c in range(NCORES)))
        for t in range(T1)
    ]
    plan1 = GatherPlan(T1, m1, NI_GATHER)
    padL = [[t * P for t in range(T1)] for _ in range(NCORES)]
    _fill_gather(plan1, tiles1, padL)
    plan1.compute_call_bounds(n0_pad)
    assert n0_pad <= WINDOW and n1_pad <= WINDOW

    # ---- layer 2: dense sweep over SBUF-resident h2 (no gather) ----
    # sp2dense[c][j] = [128, 128] mask: W[row, seed] = sum inv2 over edges
    # (src local j*128+row -> seed).
    sp2d = []
    for c in range(NCORES):
        es, ed, inv = blocks[c]["e2"]
        W = np.zeros((T1, P, P), np.float32)
        np.add.at(W, (es // P, es % P, ed), inv)
        sp2d.append(
            np.ascontiguousarray(
                W.transpose(1, 0, 2).reshape(P, T1 * P).astype(bf16)
            )
        )

    return dict(
        plan0=plan0,
        plans=(plan1,),
        sp2d=sp2d,
        T=(T0, T1, T2),
        n0_pad=n0_pad,
        n1_pad=n1_pad,
        bandd=bandd,
        bandm=bandm,
        blocks=blocks,
        weights=tuple(
            (
                np.asarray(inputs[f"W_self{l}"], np.float32),
                np.asarray(inputs[f"W_neigh{l}"], np.float32),
                np.asarray(inputs[f"b{l}"], np.float32),
            )
            for l in range(3)
        ),
    )


# ---------------------------------------------------------------------------
# Numpy simulation of the device kernel (validation aid; fp32 stand-in)
# ---------------------------------------------------------------------------


def simulate_core(meta, c):
    plan0 = meta["plan0"]
    colidx = np.arange(P, dtype=np.float32)

    mdov = plan0.mdov[c]
    ws, wn, b = meta["weights"][0]
    table = np.zeros((plan0.n_tiles * P, OUTW[0]), np.float32)
    for t in range(plan0.n_tiles):
        K, O = plan0.K[t], plan0.O[t]
        hdT = meta["bandd"][c][t].astype(np.float32)
        hd = np.concatenate([hdT[:, 0:P].T, hdT[:, P : 2 * P].T], axis=1)
        xm = meta["bandm"][c][t].astype(np.float32).reshape(P, K + O, FEAT)
        mean = xm[:, 0:K, :].sum(axis=1)  # [d, f]
        oo = int(plan0.ov_off[t])
        for o in range(O):
            msgs = xm[:, K + o, :]
            onehot = colidx[None, :] == mdov[:, oo + o : oo + o + 1]
            mean += (msgs.T @ onehot).T
        table[t * P : (t + 1) * P] = np.maximum(hd @ ws + mean @ wn + b, 0.0)

    plan = meta["plans"][0]
    md1 = plan.md[c].reshape(P, plan.n_sp_cols, 2)
    table8 = table.astype(_fp8()).astype(np.float32)  # h1buf is fp8
    ws, wn, b = meta["weights"][1]
    out = np.zeros((plan.n_tiles * P, OUTW[1]), np.float32)
    for t in range(plan.n_tiles):
        hd = table[t * P : (t + 1) * P]
        aggT = np.zeros((FEAT, P), np.float32)
        for sp_col, ch in plan.tile_pairs[t]:
            msgs = table8[plan.gidx[c][:, ch]]
            mask = (colidx[None, :] == md1[:, sp_col, 0:1]) * md1[:, sp_col, 1:2]
            aggT += msgs.T @ mask
        out[t * P : (t + 1) * P] = np.maximum(hd @ ws + aggT.T @ wn + b, 0.0)
    table = out

    # layer 2: dense sweep
    ws, wn, b = meta["weights"][2]
    sp2 = meta["sp2d"][c].astype(np.float32).reshape(P, -1, P)
    hd = table[0:P]
    aggT = np.zeros((FEAT, P), np.float32)
    for j in range(sp2.shape[1]):
        aggT += table[j * P : (j + 1) * P].T @ sp2[:, j, :]
    y = hd @ ws + aggT.T @ wn + b
    return y[:SEEDS_PER_CORE]


# ---------------------------------------------------------------------------
# Device kernel
# ---------------------------------------------------------------------------


def _wrap_idx16(plan, c):
    bases = np.zeros(plan.n_chunks_pad, np.int64)
    for k in range(plan.n_calls):
        bases[plan.call_chunk_off[k] : plan.call_chunk_off[k + 1]] = plan.call_base[k]
    rel = plan.gidx[c] - bases[None, :]
    total16 = plan.n_chunks_pad * P // 16
    out = np.zeros((P, total16), np.int16)
    off16 = 0
    for k in range(plan.n_calls):
        a, b = int(plan.call_chunk_off[k]), int(plan.call_chunk_off[k + 1])
        flat = rel[:, a:b].T.reshape(-1)
        w = flat.reshape(len(flat) // 16, 16).T.astype(np.int16)
        out[:16, off16 : off16 + w.shape[1]] = w
        off16 += w.shape[1]
    for rep in range(1, 8):
        out[rep * 16 : (rep + 1) * 16] = out[:16]
    return out


def run_device(meta, trace=False, debug_h1=False):
    import concourse.bacc as bacc
    import concourse.tile as tile
    import concourse.mybir as mybir
    from concourse.bass_utils import run_bass_kernel_spmd

    plan0 = meta["plan0"]
    plan1 = meta["plans"][0]
    T1 = meta["T"][1]
    f32 = mybir.dt.float32
    i32 = mybir.dt.int32
    b16 = mybir.dt.bfloat16
    f8 = mybir.dt.float8e4
    alu = mybir.AluOpType

    nc = bacc.Bacc("TRN2", target_bir_lowering=False, debug=False, num_devices=NCORES)

    # one DRAM tensor per band tile: the transfer is then one fully
    # contiguous HBM region (a [P, W] slice of a wide tensor reads 128
    # scattered ~7KB segments and caps at ~260 GB/s).  dst groups bf16,
    # message grids fp8 (mean-aggregated -> quantization error washes out)
    bandd_d = [
        nc.dram_tensor(f"bandd{t}", [P, FEAT], b16, kind="ExternalInput")
        for t in range(plan0.n_tiles)
    ]
    bandm_d = [
        nc.dram_tensor(
            f"bandm{t}",
            [P, (plan0.K[t] + plan0.O[t]) * FEAT],
            f8,
            kind="ExternalInput",
        )
        for t in range(plan0.n_tiles)
    ]
    mdov_d = nc.dram_tensor("mdov", [P, max(plan0.n_ov_cols, 1)], f32,
                            kind="ExternalInput")
    ident_d = nc.dram_tensor("ident", [P, P], b16, kind="ExternalInput")
    ones_d = nc.dram_tensor("ones", [1, P], b16, kind="ExternalInput")
    h1buf = nc.dram_tensor("h1buf", [meta["n0_pad"], FEAT], f8)
    out_d = nc.dram_tensor("out", [SEEDS_PER_CORE, OUTW[2]], f32, kind="ExternalOutput")

    h1o_d = None
    if debug_h1:
        h1o_d = nc.dram_tensor("h1o", [meta["n0_pad"], FEAT], f32,
                               kind="ExternalOutput")
    idx1_d = nc.dram_tensor("gidx1", [P, plan1.n_chunks_pad * P // 16],
                            mybir.dt.int16, kind="ExternalInput")
    md1_d = nc.dram_tensor("md1", [P, plan1.n_sp_cols * 2], f32,
                           kind="ExternalInput")
    sp2_d = nc.dram_tensor("sp2d", [P, T1 * P], b16, kind="ExternalInput")
    w_d = []
    for l in range(3):
        w_d.append(
            (
                nc.dram_tensor(f"ws{l}", [FEAT, OUTW[l]], b16, kind="ExternalInput"),
                nc.dram_tensor(f"wn{l}", [FEAT, OUTW[l]], b16, kind="ExternalInput"),
                nc.dram_tensor(f"bias{l}", [1, OUTW[l]], b16, kind="ExternalInput"),
            )
        )

    use_bias = [bool(np.any(meta["weights"][l][2] != 0)) for l in range(3)]

    with tile.TileContext(nc) as tc:
        with (
            tc.tile_pool(name="const", bufs=1) as cpool,
            tc.tile_pool(name="msgs", bufs=9) as mpool,
            tc.tile_pool(name="dsts", bufs=9) as dpool,
            tc.tile_pool(name="mask", bufs=28) as kpool,
            tc.tile_pool(name="acc", bufs=3) as apool,
            tc.tile_pool(name="outp", bufs=3) as opool,
            tc.tile_pool(name="hdt", bufs=3) as hpool,
            tc.tile_pool(name="gmsg", bufs=1) as gpool,
            tc.tile_pool(name="pagg", bufs=2, space="PSUM") as pa,
            tc.tile_pool(name="py", bufs=2, space="PSUM") as pypool,
        ):
            # ---- upfront consts ----
            ident_t = cpool.tile([P, P], b16, tag="ident")
            nc.scalar.dma_start(out=ident_t[:], in_=ident_d[:])
            mdov_t = cpool.tile([P, max(plan0.n_ov_cols, 1)], f32, tag="mdov")
            nc.scalar.dma_start(out=mdov_t[:], in_=mdov_d[:])
            colidx_i = cpool.tile([P, P], i32, tag="colidx_i")
            nc.gpsimd.iota(colidx_i[:], [[1, P]], channel_multiplier=0)
            colidx = cpool.tile([P, P], f32, tag="colidx")
            nc.vector.tensor_copy(out=colidx[:], in_=colidx_i[:])

            ws_ts, wn_ts, bias_ts = [[None, None] for _ in range(3)], \
                [[None, None] for _ in range(3)], [None] * 3
            ones_t = cpool.tile([1, P], b16, tag="ones")

            def load_weights(l, eng):
                outw = OUTW[l]
                for k in range(2):
                    w = cpool.tile([P, outw], b16, tag=f"ws{l}_{k}")
                    eng.dma_start(out=w[:], in_=w_d[l][0][k * P : (k + 1) * P, :])
                    ws_ts[l][k] = w
                    w = cpool.tile([P, outw], b16, tag=f"wn{l}_{k}")
                    eng.dma_start(out=w[:], in_=w_d[l][1][k * P : (k + 1) * P, :])
                    wn_ts[l][k] = w
                if use_bias[l]:
                    bias_t = cpool.tile([1, outw], b16, tag=f"bias{l}")
                    eng.dma_start(out=bias_t[:], in_=w_d[l][2][:])
                    bias_ts[l] = bias_t

            load_weights(0, nc.scalar)
            if any(use_bias):
                nc.scalar.dma_start(out=ones_t[:], in_=ones_d[:])

            h2res = [
                cpool.tile([P, FEAT], b16, tag=f"h2res_{t}", name=f"h2res_{t}")
                for t in range(T1)
            ]
            h1res = [
                cpool.tile([P, FEAT], b16, tag=f"h1res_{t}", name=f"h1res_{t}")
                for t in range(T1)
            ]

            def gen_mask(md_t, col):
                """One-hot mask [128 slots, 128 dst] = (colidx==d[p]) * inv[p]."""
                mk = kpool.tile([P, P], b16, tag="mk")
                nc.vector.tensor_scalar(
                    out=mk[:],
                    in0=colidx[:],
                    scalar1=md_t[:, 2 * col : 2 * col + 1],
                    scalar2=md_t[:, 2 * col + 1 : 2 * col + 2],
                    op0=alu.is_equal,
                    op1=alu.mult,
                )
                return mk

            def gen_mask_ov(col):
                """Pure one-hot [128 slots, 128 dst] = (colidx==d[p])."""
                mk = kpool.tile([P, P], b16, tag="mk")
                nc.vector.tensor_scalar(
                    out=mk[:],
                    in0=colidx[:],
                    scalar1=mdov_t[:, col : col + 1],
                    scalar2=None,
                    op0=alu.is_equal,
                )
                return mk

            def tile_tail(l, t, ac, hdT, dest):
                """Y matmuls + bias + activation + store for one dst tile.
                ac = aggT halves [f-half, d] bf16; hdT = dst rows transposed."""
                outw = OUTW[l]
                y = pypool.tile([P, outw], f32, tag="y")
                nc.tensor.matmul(y[:], lhsT=ac[:, 0:P], rhs=wn_ts[l][0][:],
                                 start=True, stop=False)
                nc.tensor.matmul(y[:], lhsT=ac[:, P : 2 * P], rhs=wn_ts[l][1][:],
                                 start=False, stop=False)
                nc.tensor.matmul(y[:], lhsT=hdT[:, 0:P], rhs=ws_ts[l][0][:],
                                 start=False, stop=False)
                nc.tensor.matmul(y[:], lhsT=hdT[:, P : 2 * P], rhs=ws_ts[l][1][:],
                                 start=False, stop=not use_bias[l])
                if use_bias[l]:
                    nc.tensor.matmul(y[:], lhsT=ones_t[0:1, :],
                                     rhs=bias_ts[l][0:1, :],
                                     start=False, stop=True)
                if l == 0:
                    o2 = opool.tile([P, outw], f8, tag="o2")
                    nc.scalar.activation(
                        out=o2[:], in_=y[:],
                        func=mybir.ActivationFunctionType.Relu,
                    )
                    nc.scalar.dma_start(out=dest[t * P : (t + 1) * P, :], in_=o2[:])
                    if t < T1:
                        # bf16 copy kept on-chip for the layer-1 self path
                        nc.scalar.activation(
                            out=h1res[t][:], in_=y[:],
                            func=mybir.ActivationFunctionType.Relu,
                        )
                    if debug_h1:
                        od = opool.tile([P, outw], f32, tag="od")
                        nc.vector.tensor_copy(out=od[:], in_=o2[:])
                        nc.sync.dma_start(
                            out=h1o_d[t * P : (t + 1) * P, :], in_=od[:]
                        )
                elif l == 1:
                    nc.scalar.activation(
                        out=h2res[t][:], in_=y[:],
                        func=mybir.ActivationFunctionType.Relu,
                    )
                else:
                    o = opool.tile([P, outw], f32, tag="o")
                    nc.vector.tensor_copy(out=o[:], in_=y[:])
                    nc.sync.dma_start(out=dest[:], in_=o[0:SEEDS_PER_CORE, :])

            # ================= layer 0: dense bands =================
            # 1-deep software pipeline: tile t's mask matmuls are emitted
            # before tile t-1's PSUM copy + Y matmuls, so the PE never
            # waits on the DVE copy.
            # Layer-1 mask generation (needs only md1/colidx) is spread
            # through this loop, 1-2 per tile, emitted AFTER each tile's
            # PSUM copies so it's lowest-priority DVE work -- bunching all
            # 63 at the layer boundary stalled the copies (PE dip 20-50us).
            l1_flat = [
                (t1, i, sp_col)
                for t1 in range(plan1.n_tiles)
                for i, (sp_col, _) in enumerate(plan1.tile_pairs[t1])
            ]
            l1_masks = [
                [None] * len(plan1.tile_pairs[t1])
                for t1 in range(plan1.n_tiles)
            ]
            l1_next = [0]

            def emit_l1_masks(n):
                while l1_next[0] < len(l1_flat) and n > 0:
                    t1, i, sp_col = l1_flat[l1_next[0]]
                    mk = cpool.tile([P, P], b16, tag=f"mk1_{t1}_{i}",
                                    name=f"mk1_{t1}_{i}")
                    nc.vector.tensor_scalar(
                        out=mk[:],
                        in0=colidx[:],
                        scalar1=md1_t[:, 2 * sp_col : 2 * sp_col + 1],
                        scalar2=md1_t[:, 2 * sp_col + 1 : 2 * sp_col + 2],
                        op0=alu.is_equal,
                        op1=alu.mult,
                    )
                    l1_masks[t1][i] = mk
                    l1_next[0] += 1
                    n -= 1

            Kmax = max(k + o for k, o in zip(plan0.K, plan0.O))
            pending = None  # (t, pc0, pc1, hdT_view)
            for t in range(plan0.n_tiles):
                K, O = plan0.K[t], plan0.O[t]
                oo = int(plan0.ov_off[t])
                btd = dpool.tile([P, FEAT], b16, tag="bandd")
                nc.sync.dma_start(out=btd[:], in_=bandd_d[t][:])
                btm = mpool.tile([P, Kmax * FEAT], f8, tag="bandm")
                nc.sync.dma_start(
                    out=btm[:, : (K + O) * FEAT], in_=bandm_d[t][:]
                )
                omasks = [gen_mask_ov(oo + o) for o in range(O)]
                # two PSUM tiles: each accumulation group must own its bank
                # (start=True resets the whole bank's has_written); rhs is
                # the constant identity for grid chunks (rows pre-scaled,
                # dst-aligned), a one-hot mask for overflow chunks
                pc0 = pa.tile([P, P], f32, tag="pc0")
                pc1 = pa.tile([P, P], f32, tag="pc1")
                for k in range(K + O):
                    st, sp = (k == 0), (k == K + O - 1)
                    base = k * FEAT
                    rhs = ident_t[:] if k < K else omasks[k - K][:]
                    nc.tensor.matmul(pc0[:], lhsT=btm[:, base : base + P],
                                     rhs=rhs, start=st, stop=sp)
                    nc.tensor.matmul(pc1[:],
                                     lhsT=btm[:, base + P : base + 2 * P],
                                     rhs=rhs, start=st, stop=sp)
                if pending is not None:
                    tp, pc0p, pc1p, hdTp = pending
                    ac = apool.tile([P, FEAT], b16, tag="ac")
                    nc.vector.tensor_copy(out=ac[:, 0:P], in_=pc0p[:])
                    nc.vector.tensor_copy(out=ac[:, P : 2 * P], in_=pc1p[:])
                    tile_tail(0, tp, ac, hdTp, h1buf)
                if t >= 2:
                    emit_l1_masks(1 if t < 30 else 2)
                pending = (t, pc0, pc1, btd[:])

                if t == 0:
                    # late consts: emitted behind the first band loads so the
                    # main stream starts immediately; md1 first (the l1 mask
                    # generation spread through this loop needs it by t=2).
                    md1_t = cpool.tile([P, plan1.n_sp_cols * 2], f32, tag="md1")
                    nc.scalar.dma_start(out=md1_t[:], in_=md1_d[:])
                    idx1_t = cpool.tile(list(idx1_d.shape), mybir.dt.int16,
                                        tag="idx1")
                    nc.scalar.dma_start(out=idx1_t[:], in_=idx1_d[:])
                    load_weights(1, nc.scalar)
                    load_weights(2, nc.scalar)
                    sp2_t = cpool.tile([P, T1 * P], b16, tag="sp2d")
                    nc.scalar.dma_start(out=sp2_t[:], in_=sp2_d[:])
            # flush the pipeline
            tp, pc0p, pc1p, hdTp = pending
            ac = apool.tile([P, FEAT], b16, tag="ac")
            nc.vector.tensor_copy(out=ac[:, 0:P], in_=pc0p[:])
            nc.vector.tensor_copy(out=ac[:, P : 2 * P], in_=pc1p[:])
            tile_tail(0, tp, ac, hdTp, h1buf)

            # ================= layer 1: overlapped gather =================
            # flush any layer-1 masks not yet emitted in the loop above
            emit_l1_masks(len(l1_flat))

            # transposed h_dst tiles via identity matmuls from the
            # SBUF-resident h1res copies (no DMA-ring blocking)
            hdts = []
            for t in range(T1):
                ph = pa.tile([P, FEAT], f32, tag="pht", name=f"pht_{t}")
                nc.tensor.matmul(ph[:, 0:P], lhsT=h1res[t][:, 0:P],
                                 rhs=ident_t[:], start=True, stop=True)
                nc.tensor.matmul(ph[:, P : 2 * P], lhsT=h1res[t][:, P : 2 * P],
                                 rhs=ident_t[:], start=True, stop=True)
                ht = hpool.tile([P, FEAT], b16, tag=f"hdt_{t}", name=f"hdt_{t}")
                nc.vector.tensor_copy(out=ht[:], in_=ph[:])
                hdts.append(ht)

            call_tiles = [None] * plan1.n_calls
            order = sorted(
                range(plan1.n_calls),
                key=lambda k: (int(plan1.call_hi[k]), int(plan1.call_base[k])),
            )
            for k in order:
                a = int(plan1.call_chunk_off[k])
                b2 = int(plan1.call_chunk_off[k + 1])
                sz = b2 - a
                lo = int(plan1.call_base[k])
                hi = int(plan1.call_hi[k])
                mt = gpool.tile([P, sz * FEAT], f8, tag=f"msgs1_{k}")
                nc.gpsimd.dma_gather(
                    out_ap=mt[:, : sz * FEAT].rearrange("p (g d) -> p g d", g=sz),
                    in_ap=h1buf[lo:hi, :],
                    idxs_ap=idx1_t[:, a * P // 16 : b2 * P // 16],
                    num_idxs=sz * P,
                    num_idxs_reg=sz * P,
                    elem_size=FEAT,
                    single_packet=False,
                )
                call_tiles[k] = (mt, a)

            call_of_chunk = np.searchsorted(
                plan1.call_chunk_off, np.arange(plan1.n_chunks_pad), side="right"
            ) - 1

            def msg_slice(ch, f0, f1):
                k = int(call_of_chunk[ch])
                mt, a = call_tiles[k]
                j = ch - a
                return mt[:, j * FEAT + f0 : j * FEAT + f1]

            pending = None
            for t in range(plan1.n_tiles):
                pairs = plan1.tile_pairs[t]
                masks = l1_masks[t]
                pc0 = pa.tile([P, P], f32, tag="pc0")
                pc1 = pa.tile([P, P], f32, tag="pc1")
                for i, (sp_col, ch) in enumerate(pairs):
                    st, sp = (i == 0), (i == len(pairs) - 1)
                    nc.tensor.matmul(pc0[:], lhsT=msg_slice(ch, 0, P),
                                     rhs=masks[i][:], start=st, stop=sp)
                    nc.tensor.matmul(pc1[:],
                                     lhsT=msg_slice(ch, P, 2 * P),
                                     rhs=masks[i][:], start=st, stop=sp)
                if pending is not None:
                    tp, pc0p, pc1p = pending
                    ac = apool.tile([P, FEAT], b16, tag="ac")
                    nc.vector.tensor_copy(out=ac[:, 0:P], in_=pc0p[:])
                    nc.vector.tensor_copy(out=ac[:, P : 2 * P], in_=pc1p[:])
                    tile_tail(1, tp, ac, hdts[tp], None)
                pending = (t, pc0, pc1)
            tp, pc0p, pc1p = pending
            ac = apool.tile([P, FEAT], b16, tag="ac")
            nc.vector.tensor_copy(out=ac[:, 0:P], in_=pc0p[:])
            nc.vector.tensor_copy(out=ac[:, P : 2 * P], in_=pc1p[:])
            tile_tail(1, tp, ac, hdts[tp], None)

            # ================= layer 2: dense sweep over h2res =================
            pc0 = pa.tile([P, P], f32, tag="pc0")
            pc1 = pa.tile([P, P], f32, tag="pc1")
            for j in range(T1):
                st, sp = (j == 0), (j == T1 - 1)
                nc.tensor.matmul(pc0[:], lhsT=h2res[j][:, 0:P],
                                 rhs=sp2_t[:, j * P : (j + 1) * P],
                                 start=st, stop=sp)
                nc.tensor.matmul(pc1[:], lhsT=h2res[j][:, P : 2 * P],
                                 rhs=sp2_t[:, j * P : (j + 1) * P],
                                 start=st, stop=sp)
            # transpose h2res[0] for the self path (identity matmuls)
            ph0 = pa.tile([P, P], f32, tag="pc0")
            ph1 = pa.tile([P, P], f32, tag="pc1")
            nc.tensor.matmul(ph0[:], lhsT=h2res[0][:, 0:P],
                             rhs=ident_t[:], start=True, stop=True)
            nc.tensor.matmul(ph1[:], lhsT=h2res[0][:, P : 2 * P],
                             rhs=ident_t[:], start=True, stop=True)
            hdT2 = apool.tile([P, FEAT], b16, tag="hdT2")
            nc.vector.tensor_copy(out=hdT2[:, 0:P], in_=ph0[:])
            nc.vector.tensor_copy(out=hdT2[:, P : 2 * P], in_=ph1[:])
            ac = apool.tile([P, FEAT], b16, tag="ac")
            nc.vector.tensor_copy(out=ac[:, 0:P], in_=pc0[:])
            nc.vector.tensor_copy(out=ac[:, P : 2 * P], in_=pc1[:])
            tile_tail(2, 0, ac, hdT2, out_d)

    nc.compile()

    in_maps = []
    bf16 = _bf16()
    eye16 = np.eye(P, dtype=bf16)
    for c in range(NCORES):
        m = dict(
            mdov=plan0.mdov[c],
            ident=eye16,
            ones=np.ones((1, P), dtype=bf16),
            gidx1=_wrap_idx16(plan1, c),
            md1=plan1.md[c],
            sp2d=meta["sp2d"][c],
        )
        for t in range(plan0.n_tiles):
            m[f"bandd{t}"] = meta["bandd"][c][t]
            m[f"bandm{t}"] = meta["bandm"][c][t]
        for l in range(3):
            ws, wn, b = meta["weights"][l]
            m[f"ws{l}"] = np.ascontiguousarray(ws.astype(bf16))
            m[f"wn{l}"] = np.ascontiguousarray(wn.astype(bf16))
            m[f"bias{l}"] = np.ascontiguousarray(b[None, :].astype(bf16))
        in_maps.append(m)

    res = run_bass_kernel_spmd(
        nc, in_maps, core_ids=list(range(NCORES)), trace=trace
    )
    if debug_h1:
        return [res.results[c]["out"] for c in range(NCORES)], res, [
            res.results[c]["h1o"] for c in range(NCORES)
        ]
    return [res.results[c]["out"] for c in range(NCORES)], res


def assemble(meta, outs):
    full = np.zeros((NUM_DST[2], OUTW[2]), np.float32)
    for c in range(NCORES):
        full[meta["blocks"][c]["seeds"]] = outs[c]
    return full


def kernel(**inputs) -> np.ndarray:
    meta = build_host(inputs)
    outs, _ = run_device(meta)
    return assemble(meta, outs)
